# revision 13
# baseline (speedup 1.0000x reference)
"""CPI-MPNN (molecule MPNN + protein CNN + FC head) Trainium2 kernel.

Self-contained: hardcodes all shapes. Shards the batch (128) across 8
NeuronCores (16 samples each), replicates the small weights.

Strategy:
  - Host (numpy): protein embedding gather (-> channel-major, fp8 e4m3
    with power-of-2 scaling, conv pads + the tap-shifted second plane
    baked in), bond/atom graph one-hot adjacency matrices (gather+sum
    == matmul), weight transposes/padding.
  - Device: conv tower in fp8 e4m3 with DoubleRow matmuls (2 taps per
    pass -> ~half the PE streaming cycles; end-to-end quantization err
    ~1e-3 vs the 2e-2 gate), MPNN in float32r (full PE rate at N>=256,
    ~1.5e-4 per matmul). Conv1d = per-tap-pair matmuls accumulated in
    PSUM over a zero-padded two-plane activation layout (plane1 =
    plane0 shifted one position, so one DoubleRow pass covers taps
    (d, d+1)); maxpool moved before bias+relu (monotonicity). MPNN per
    molecule with PE transposes for the W_h contraction. Emission is
    stage-interleaved and DMA queue order is tuned so the in-order PE
    stream always has data-ready work.
"""

import numpy as np
from contextlib import ExitStack

import concourse.bass as bass
import concourse.tile as tile
from concourse import bacc, mybir
from concourse.bass_utils import run_bass_kernel_spmd
from concourse.masks import make_identity

F32 = mybir.dt.float32
F32R = mybir.dt.float32r
BF16 = mybir.dt.bfloat16
FP8 = mybir.dt.float8e4
DR = mybir.MatmulPerfMode.DoubleRow
AF = mybir.ActivationFunctionType
ALU = mybir.AluOpType

# model dims
H = 200
ATOM_FDIM = 39
BOND_FDIM = 11
B, NA, NB = 128, 48, 96
L, VOCAB = 1000, 26
KERNALS = [3, 5, 7]
PF = [50, 96, 128, 200]
FC_DIMS = [400, 200, 100, 1]

NCORES = 8
M = B // NCORES          # molecules per core (16)
SEG = 1008               # 3 + 1000 + 5 padded segment (16B-aligned plane)
PAD = 3
NCH = 500                # conv free-dim chunk (2 per sample)

# fp8 power-of-2 scales (exactly cancelled in the psum->act epilogues)
S0 = 128.0               # protein embedding activations
SW = 128.0               # conv weights
S1 = 256.0               # x1 activations
S2 = 256.0               # x2 activations
ACT0_SCALE = S1 / (S0 * SW)    # 2^-6
ACT1_SCALE = S2 / (S1 * SW)    # 2^-7
FIN_SCALE = 1.0 / (S2 * SW)    # 2^-15

_CACHE = {}


def _build_nc():
    nc = bacc.Bacc("TRN2", target_bir_lowering=False, debug=False)

    # ---- DRAM inputs (per core) ----
    # protein activations go over the wire in fp8 (scaled by S0) as two
    # planes: plane1 = plane0 shifted one position, so a DoubleRow AP
    # over both planes covers the tap pair (d, d+1).
    d_pvt = [nc.dram_tensor(f"pvt{g}", [50, 2, SEG], FP8, kind="ExternalInput")
             for g in range(M)]
    d_fbt = nc.dram_tensor("fbt", [50, M, 96], F32R, kind="ExternalInput")
    d_cat1 = nc.dram_tensor("cat1", [40, M, 48], F32R, kind="ExternalInput")
    # adjacency counts are small integers: exact in bf16 on the wire,
    # converted to f32r on-chip (DVE) for the fp32r matmuls
    d_abt = nc.dram_tensor("abt", [96, M, 96], BF16, kind="ExternalInput")
    d_aat = nc.dram_tensor("aat", [96, M, 48], BF16, kind="ExternalInput")

    d_wi = nc.dram_tensor("wi", [50, 256], F32R, kind="ExternalInput")
    d_wh = nc.dram_tensor("wh", [100, 2, 256], F32R, kind="ExternalInput")
    d_wo1 = nc.dram_tensor("wo1", [40, 256], F32R, kind="ExternalInput")
    d_wo2 = nc.dram_tensor("wo2", [128, 256], F32R, kind="ExternalInput")
    d_wo3 = nc.dram_tensor("wo3", [72, 256], F32R, kind="ExternalInput")
    # conv weights, fp8: DoubleRow tap-pair packs + the odd single tap
    d_w0dr = nc.dram_tensor("w0dr", [50, 2, 96], FP8, kind="ExternalInput")
    d_w0s = nc.dram_tensor("w0s", [50, 96], FP8, kind="ExternalInput")
    d_b0 = nc.dram_tensor("b0", [96, 1], F32, kind="ExternalInput")
    d_w1dr = nc.dram_tensor("w1dr", [96, 4, 128], FP8, kind="ExternalInput")
    d_w1s = nc.dram_tensor("w1s", [96, 128], FP8, kind="ExternalInput")
    d_b1 = nc.dram_tensor("b1", [128, 1], F32, kind="ExternalInput")
    d_w2dra = nc.dram_tensor("w2dra", [128, 6, 128], FP8, kind="ExternalInput")
    d_w2sa = nc.dram_tensor("w2sa", [128, 128], FP8, kind="ExternalInput")
    # 72-col group padded to 80 so the DoubleRow plane step is 16B-aligned
    d_w2drb = nc.dram_tensor("w2drb", [128, 6, 80], FP8, kind="ExternalInput")
    d_w2sb = nc.dram_tensor("w2sb", [128, 72], FP8, kind="ExternalInput")
    d_b2a = nc.dram_tensor("b2a", [128, 1], F32, kind="ExternalInput")
    d_b2b = nc.dram_tensor("b2b", [72, 1], F32, kind="ExternalInput")
    d_fc0 = [nc.dram_tensor(f"fc0{k}", [dim, 200], F32R, kind="ExternalInput")
             for k, dim in (("a", 128), ("b", 72), ("c", 128), ("d", 72))]
    d_fc0ba = nc.dram_tensor("fc0ba", [128, 1], F32, kind="ExternalInput")
    d_fc0bb = nc.dram_tensor("fc0bb", [72, 1], F32, kind="ExternalInput")
    d_fc1a = nc.dram_tensor("fc1a", [128, 100], F32R, kind="ExternalInput")
    d_fc1b = nc.dram_tensor("fc1b", [72, 100], F32R, kind="ExternalInput")
    d_fc1bias = nc.dram_tensor("fc1bias", [100, 1], F32, kind="ExternalInput")
    d_fc2w = nc.dram_tensor("fc2w", [100, 1], F32R, kind="ExternalInput")
    d_fc2b = nc.dram_tensor("fc2b", [1, 1], F32, kind="ExternalInput")
    d_ones = nc.dram_tensor("ones48", [48, 1], F32R, kind="ExternalInput")

    d_out = nc.dram_tensor("out", [1, M], F32, kind="ExternalOutput")

    with tile.TileContext(nc) as tc, ExitStack() as ctx:
        cst = ctx.enter_context(tc.tile_pool(name="cst", bufs=1))
        sbs = ctx.enter_context(tc.tile_pool(name="sbs", bufs=1))   # static per-mol state
        tmp = ctx.enter_context(tc.tile_pool(name="tmp", bufs=1))
        xp = ctx.enter_context(tc.tile_pool(name="xp", bufs=1))
        pp = ctx.enter_context(tc.tile_pool(name="pp", bufs=1, space="PSUM"))

        # ---- load constants ----
        # DMA issue order matters: the critical path at kernel start is
        # (wi, fbt, abt) for the MPNN and (w0, b0, pvt0) for the conv.
        # Spread issue across three HWDGE engines (SP/ACT/DVE are idle).
        def const_tile(dram, shape, dtype=F32R, name=None, eng=None):
            t = cst.tile(shape, dtype, tag=name or dram.name)
            (eng or nc.sync).dma_start(t[:], dram.ap())
            return t

        ident = cst.tile([128, 128], F32, tag="ident")
        make_identity(nc, ident[:])

        # MPNN inputs arrive in 4 molecule-groups so mol 0 isn't gated on
        # the whole batch. Group g covers mols 4g..4g+3. Adjacency counts
        # come in bf16 (exact) and are converted to f32r by the DVE.
        GM = 4
        fbt_g, abt_g, aat_g, cat1_g = {}, {}, {}, {}

        def fbt_dma(g):
            t = cst.tile([50, GM * 96], F32R, tag=f"fbt{g}")
            nc.sync.dma_start(t[:].rearrange("p (m i) -> p m i", i=96),
                              d_fbt.ap()[:, GM * g:GM * (g + 1), :])
            fbt_g[g] = t

        def abt_dma(g):
            r = cst.tile([96, GM * 96], BF16, tag=f"abtr{g}")
            nc.sync.dma_start(r[:].rearrange("p (m i) -> p m i", i=96),
                              d_abt.ap()[:, GM * g:GM * (g + 1), :])
            t = cst.tile([96, GM * 96], F32R, tag=f"abt{g}")
            nc.vector.tensor_copy(t[:], r[:])
            abt_g[g] = t

        def aat_cat_dma(g, eng):
            r = cst.tile([96, GM * 48], BF16, tag=f"aatr{g}")
            eng.dma_start(r[:].rearrange("p (m i) -> p m i", i=48),
                          d_aat.ap()[:, GM * g:GM * (g + 1), :])
            t = cst.tile([96, GM * 48], F32R, tag=f"aat{g}")
            nc.vector.tensor_copy(t[:], r[:])
            aat_g[g] = t
            t = cst.tile([40, GM * 48], F32R, tag=f"cat1{g}")
            eng.dma_start(t[:].rearrange("p (m i) -> p m i", i=48),
                          d_cat1.ap()[:, GM * g:GM * (g + 1), :])
            cat1_g[g] = t

        # SP queue, landing-time tuned (~22 GB/s per HWDGE queue):
        # mol-group-0 first, then wh / conv2 weights, then later groups.
        wi_t = const_tile(d_wi, [50, 256])
        fbt_dma(0)
        abt_dma(0)
        wh_t = cst.tile([100, 2 * 256], F32R, tag="wh")
        nc.sync.dma_start(wh_t[:].rearrange("p (c n) -> p c n", n=256),
                          d_wh.ap())
        w2dra_t = cst.tile([128, 6 * 128], FP8, tag="w2dra")
        nc.sync.dma_start(w2dra_t[:].rearrange("p (t o) -> p t o", o=128),
                          d_w2dra.ap())
        w2sa_t = const_tile(d_w2sa, [128, 128], FP8)
        w2drb_t = cst.tile([128, 6 * 80], FP8, tag="w2drb")
        nc.sync.dma_start(w2drb_t[:].rearrange("p (t o) -> p t o", o=80),
                          d_w2drb.ap())
        w2sb_t = const_tile(d_w2sb, [128, 72], FP8)
        aat_cat_dma(0, nc.sync)
        wo1_t = const_tile(d_wo1, [40, 256], eng=nc.sync)
        wo2_t = const_tile(d_wo2, [128, 256], eng=nc.sync)
        wo3_t = const_tile(d_wo3, [72, 256], eng=nc.sync)
        ones_t = const_tile(d_ones, [48, 1], eng=nc.sync)
        b2a_t = const_tile(d_b2a, [128, 1], F32, eng=nc.sync)
        b2b_t = const_tile(d_b2b, [72, 1], F32, eng=nc.sync)
        fbt_dma(1)
        abt_dma(1)
        aat_cat_dma(1, nc.sync)
        fbt_dma(2)
        abt_dma(2)
        aat_cat_dma(2, nc.sync)
        fbt_dma(3)
        abt_dma(3)
        aat_cat_dma(3, nc.sync)
        fc0_t = [const_tile(d, [dim, 200], eng=nc.sync) for d, dim in
                 zip(d_fc0, (128, 72, 128, 72))]
        fc0ba_t = const_tile(d_fc0ba, [128, 1], F32, eng=nc.sync)
        fc0bb_t = const_tile(d_fc0bb, [72, 1], F32, eng=nc.sync)
        fc1a_t = const_tile(d_fc1a, [128, 100], eng=nc.sync)
        fc1b_t = const_tile(d_fc1b, [72, 100], eng=nc.sync)
        fc1bias_t = const_tile(d_fc1bias, [100, 1], F32, eng=nc.sync)
        fc2w_t = const_tile(d_fc2w, [100, 1], eng=nc.sync)
        fc2b_t = const_tile(d_fc2b, [1, 1], F32, eng=nc.sync)

        # ACT queue: conv weights + per-sample x0 buffers + atom weights
        w0dr_t = cst.tile([50, 2 * 96], FP8, tag="w0dr")
        nc.scalar.dma_start(w0dr_t[:].rearrange("p (t o) -> p t o", o=96),
                            d_w0dr.ap())
        w0s_t = const_tile(d_w0s, [50, 96], FP8, eng=nc.scalar)
        b0_t = const_tile(d_b0, [96, 1], F32, eng=nc.scalar)
        b1_t = const_tile(d_b1, [128, 1], F32, eng=nc.scalar)
        x0_bufs = []

        def pvt_dma(s):
            t = xp.tile([50, 2 * SEG], FP8, tag=f"x0s{s}")
            nc.scalar.dma_start(t[:].rearrange("p (k c) -> p k c", k=2),
                                d_pvt[s].ap())
            x0_bufs.append(t)

        w1dr_t = cst.tile([96, 4 * 128], FP8, tag="w1dr")
        nc.scalar.dma_start(w1dr_t[:].rearrange("p (t o) -> p t o", o=128),
                            d_w1dr.ap())
        w1s_t = const_tile(d_w1s, [96, 128], FP8, eng=nc.scalar)
        pvt_dma(0)
        pvt_dma(1)
        # remaining pvt loads are issued inside emit_sample_front so the
        # ACT sequencer isn't blocked issuing DMAs ahead of its compute.

        # static outputs of the two towers, feature-major [feat, M]
        embT1 = sbs.tile([128, M], F32R, tag="embT1")
        embT2 = sbs.tile([72, M], F32R, tag="embT2")
        prT1p = sbs.tile([128, M], F32, tag="prT1p")
        prT2p = sbs.tile([72, M], F32, tag="prT2p")

        # ================= per-molecule MPNN (staged) =================
        mol_state = {}

        def emit_binput(m):
            g, r = m // GM, m % GM
            fb_m = fbt_g[g][:, r * 96:(r + 1) * 96]
            psA = pp.tile([96, 256], F32, tag="mp", bufs=3)
            nc.tensor.matmul(psA[:], fb_m, wi_t[:], start=True, stop=True)
            binp = sbs.tile([96, 200], F32, tag=f"binp{m}")
            nc.scalar.copy(binp[:], psA[:, 0:200])
            msg = sbs.tile([96, 256], F32R, tag=f"msg{m}")
            nc.gpsimd.memset(msg[:, 200:256].bitcast(F32), 0.0)
            nc.vector.tensor_scalar(msg[:, 0:200], psA[:, 0:200], 0.0, None,
                                    op0=ALU.max)
            mol_state[m] = (binp, msg)

        def emit_iter_pre(m):
            g, r = m // GM, m % GM
            ab_m = abt_g[g][:, r * 96:(r + 1) * 96]
            binp, msg = mol_state[m]
            psN = pp.tile([96, 256], F32, tag="mp", bufs=3)
            nc.tensor.matmul(psN[:], ab_m, msg[:], start=True, stop=True)
            neis = tmp.tile([96, 200], F32, tag="neis", bufs=5)
            nc.scalar.copy(neis[:], psN[:, 0:200])
            nTa = tmp.tile([100, 96], F32R, tag="nTa", bufs=6)
            nTb = tmp.tile([100, 96], F32R, tag="nTb", bufs=6)
            for c, dst in ((0, nTa), (1, nTb)):
                pt = pp.tile([100, 96], F32, tag="tp", bufs=2)
                nc.tensor.transpose(pt[:], neis[:, c * 100:(c + 1) * 100],
                                    ident[0:96, 0:96])
                nc.scalar.copy(dst[:], pt[:])
            mol_state[m] = (binp, msg, nTa, nTb)

        def emit_iter_post(m):
            binp, msg, nTa, nTb = mol_state[m]
            psH = pp.tile([96, 256], F32, tag="mp", bufs=3)
            nc.tensor.matmul(psH[:], nTa[:], wh_t[:, 0:256],
                             start=True, stop=False)
            nc.tensor.matmul(psH[:], nTb[:], wh_t[:, 256:512],
                             start=False, stop=True)
            tm = tmp.tile([96, 200], F32, tag="mtmp", bufs=3)
            nc.vector.tensor_add(tm[:], psH[:, 0:200], binp[:])
            nc.scalar.activation(msg[:, 0:200], tm[:], AF.Relu)
            mol_state[m] = (binp, msg)

        def emit_atom(m):
            g, r = m // GM, m % GM
            aa_m = aat_g[g][:, r * 48:(r + 1) * 48]
            c1_m = cat1_g[g][:, r * 48:(r + 1) * 48]
            binp, msg = mol_state[m]
            psT1 = pp.tile([128, 48], F32, tag="tp", bufs=2)
            nc.tensor.matmul(psT1[:], msg[:, 0:128], aa_m, start=True, stop=True)
            nat1 = tmp.tile([128, 48], F32R, tag="nat1", bufs=3)
            nc.scalar.copy(nat1[:], psT1[:])
            psT2 = pp.tile([72, 48], F32, tag="tp", bufs=2)
            nc.tensor.matmul(psT2[:], msg[:, 128:200], aa_m, start=True, stop=True)
            nat2 = tmp.tile([72, 48], F32R, tag="nat2", bufs=3)
            nc.scalar.copy(nat2[:], psT2[:])

            psAH = pp.tile([48, 256], F32, tag="mp", bufs=3)
            nc.tensor.matmul(psAH[:], c1_m, wo1_t[:], start=True, stop=False)
            nc.tensor.matmul(psAH[:], nat1[:], wo2_t[:], start=False, stop=False)
            nc.tensor.matmul(psAH[:], nat2[:], wo3_t[:], start=False, stop=True)
            reluh = tmp.tile([48, 200], F32R, tag="reluh", bufs=3)
            nc.scalar.activation(reluh[:], psAH[:, 0:200], AF.Relu)

            psE1 = pp.tile([128, 1], F32, tag="tp", bufs=2)
            nc.tensor.matmul(psE1[:], reluh[:, 0:128].bitcast(F32),
                             ones_t[:].bitcast(F32), start=True, stop=True)
            nc.scalar.mul(embT1[:, m:m + 1], psE1[:], 1.0 / 48)
            psE2 = pp.tile([72, 1], F32, tag="tp", bufs=2)
            nc.tensor.matmul(psE2[:], reluh[:, 128:200].bitcast(F32),
                             ones_t[:].bitcast(F32), start=True, stop=True)
            nc.scalar.mul(embT2[:, m:m + 1], psE2[:], 1.0 / 48)

        # ================= per-sample protein conv tower =================
        sample_state = {}

        # x buffers hold two fp8 planes; plane1 = plane0 shifted one
        # position left, so a DoubleRow AP with base offset o covers the
        # tap pair (x[p+o-PAD], x[p+o-PAD+1]). Data lives at plane0 cols
        # [PAD, PAD+1000); plane1 cols [PAD-1, PAD+999).
        def pads(x3):
            nc.gpsimd.memset(x3[:, 0, 0:PAD], 0.0)
            nc.gpsimd.memset(x3[:, 0, PAD + 1000:SEG], 0.0)
            nc.gpsimd.memset(x3[:, 1, 0:PAD - 1], 0.0)
            nc.gpsimd.memset(x3[:, 1, PAD + 999:SEG], 0.0)

        def emit_sample_front(s):
            if s + 2 < M:
                pvt_dma(s + 2)
            x03 = x0_bufs[s][:].rearrange("p (k c) -> p k c", k=2)
            w0dr3 = w0dr_t[:].rearrange("p (k o) -> p k o", o=96)
            x1 = xp.tile([96, 2 * SEG], FP8, tag="x1", bufs=3)
            x13 = x1[:].rearrange("p (k c) -> p k c", k=2)
            pads(x13)
            for c in range(2):
                o = c * NCH
                ps = pp.tile([96, NCH], F32, tag="cv", bufs=3)
                nc.tensor.matmul(ps[:], w0dr3, x03[:, :, o + 2:o + 2 + NCH],
                                 start=True, stop=False, perf_mode=DR)
                nc.tensor.matmul(ps[:], w0s_t[:], x03[:, 0, o + 4:o + 4 + NCH],
                                 start=False, stop=True)
                nc.scalar.activation(x13[:, 0, PAD + o:PAD + o + NCH], ps[:],
                                     AF.Relu, bias=b0_t[:], scale=ACT0_SCALE)
                nc.scalar.activation(x13[:, 1, PAD - 1 + o:PAD - 1 + o + NCH],
                                     ps[:], AF.Relu, bias=b0_t[:],
                                     scale=ACT0_SCALE)

            w1dr3 = w1dr_t[:].rearrange("p (t o) -> p t o", o=128)
            x2 = xp.tile([128, 2 * SEG], FP8, tag="x2", bufs=4)
            x23 = x2[:].rearrange("p (k c) -> p k c", k=2)
            pads(x23)
            for c in range(2):
                o = c * NCH
                ps = pp.tile([128, NCH], F32, tag="cv", bufs=3)
                nc.tensor.matmul(ps[:], w1dr3[:, 0:2, :],
                                 x13[:, :, o + 1:o + 1 + NCH],
                                 start=True, stop=False, perf_mode=DR)
                nc.tensor.matmul(ps[:], w1dr3[:, 2:4, :],
                                 x13[:, :, o + 3:o + 3 + NCH],
                                 start=False, stop=False, perf_mode=DR)
                nc.tensor.matmul(ps[:], w1s_t[:], x13[:, 0, o + 5:o + 5 + NCH],
                                 start=False, stop=True)
                nc.scalar.activation(x23[:, 0, PAD + o:PAD + o + NCH], ps[:],
                                     AF.Relu, bias=b1_t[:], scale=ACT1_SCALE)
                nc.scalar.activation(x23[:, 1, PAD - 1 + o:PAD - 1 + o + NCH],
                                     ps[:], AF.Relu, bias=b1_t[:],
                                     scale=ACT1_SCALE)
            sample_state[s] = x2

        def emit_sample_back(s):
            x2 = sample_state.pop(s)
            x23 = x2[:].rearrange("p (k c) -> p k c", k=2)
            w2dra3 = w2dra_t[:].rearrange("p (t o) -> p t o", o=128)
            w2drb3 = w2drb_t[:].rearrange("p (t o) -> p t o", o=80)
            mxA = tmp.tile([128, 2], F32, tag="mxA", bufs=3)
            mxB = tmp.tile([72, 2], F32, tag="mxB", bufs=3)
            for c in range(2):
                o = c * NCH
                psA = pp.tile([128, NCH], F32, tag="cv", bufs=3)
                for p in range(3):
                    nc.tensor.matmul(psA[:], w2dra3[:, 2 * p:2 * p + 2, :],
                                     x23[:, :, o + 2 * p:o + 2 * p + NCH],
                                     start=(p == 0), stop=False, perf_mode=DR)
                nc.tensor.matmul(psA[:], w2sa_t[:],
                                 x23[:, 0, o + 6:o + 6 + NCH],
                                 start=False, stop=True)
                nc.vector.reduce_max(mxA[:, c:c + 1], psA[:],
                                     axis=mybir.AxisListType.X)
                psB = pp.tile([72, NCH], F32, tag="cv", bufs=3)
                for p in range(3):
                    nc.tensor.matmul(psB[:], w2drb3[:, 2 * p:2 * p + 2, 0:72],
                                     x23[:, :, o + 2 * p:o + 2 * p + NCH],
                                     start=(p == 0), stop=False, perf_mode=DR)
                nc.tensor.matmul(psB[:], w2sb_t[:],
                                 x23[:, 0, o + 6:o + 6 + NCH],
                                 start=False, stop=True)
                nc.vector.reduce_max(mxB[:, c:c + 1], psB[:],
                                     axis=mybir.AxisListType.X)
            nc.vector.reduce_max(prT1p[:, s:s + 1], mxA[:],
                                 axis=mybir.AxisListType.X)
            nc.vector.reduce_max(prT2p[:, s:s + 1], mxB[:],
                                 axis=mybir.AxisListType.X)

        # Stage-interleaved emission: conv stages slot between MPNN
        # stages so the in-order PE stream always has DMA-ready filler.
        for g in range(4):
            mols = [GM * g + r for r in range(GM)]
            for m in mols:
                emit_binput(m)
            for m in mols:
                emit_iter_pre(m)
            emit_sample_front(4 * g + 0)
            emit_sample_front(4 * g + 1)
            for m in mols:
                emit_iter_post(m)
            for m in mols:
                emit_iter_pre(m)
            emit_sample_back(4 * g + 0)
            for m in mols:
                emit_iter_post(m)
            emit_sample_front(4 * g + 2)
            emit_sample_back(4 * g + 1)
            for m in mols:
                emit_atom(m)
            emit_sample_front(4 * g + 3)
            emit_sample_back(4 * g + 2)
            emit_sample_back(4 * g + 3)

        # maxpool -> descale -> bias -> relu (monotone, pool-first exact)
        prT1 = sbs.tile([128, M], F32R, tag="prT1")
        nc.scalar.activation(prT1[:], prT1p[:], AF.Relu, bias=b2a_t[:],
                             scale=FIN_SCALE)
        prT2 = sbs.tile([72, M], F32R, tag="prT2")
        nc.scalar.activation(prT2[:], prT2p[:], AF.Relu, bias=b2b_t[:],
                             scale=FIN_SCALE)

        # ================= FC head =================
        rhs4 = (embT1, embT2, prT1, prT2)
        ps0a = pp.tile([128, M], F32, tag="tp", bufs=2)
        for k in range(4):
            nc.tensor.matmul(ps0a[:], fc0_t[k][:, 0:128], rhs4[k][:],
                             start=(k == 0), stop=(k == 3))
        h0a = tmp.tile([128, M], F32R, tag="h0a")
        nc.scalar.activation(h0a[:], ps0a[:], AF.Relu, bias=fc0ba_t[:])
        ps0b = pp.tile([72, M], F32, tag="tp", bufs=2)
        for k in range(4):
            nc.tensor.matmul(ps0b[:], fc0_t[k][:, 128:200], rhs4[k][:],
                             start=(k == 0), stop=(k == 3))
        h0b = tmp.tile([72, M], F32R, tag="h0b")
        nc.scalar.activation(h0b[:], ps0b[:], AF.Relu, bias=fc0bb_t[:])

        ps1 = pp.tile([100, M], F32, tag="tp", bufs=2)
        nc.tensor.matmul(ps1[:], fc1a_t[:], h0a[:], start=True, stop=False)
        nc.tensor.matmul(ps1[:], fc1b_t[:], h0b[:], start=False, stop=True)
        h1 = tmp.tile([100, M], F32R, tag="h1")
        nc.scalar.activation(h1[:], ps1[:], AF.Relu, bias=fc1bias_t[:])

        ps2 = pp.tile([1, M], F32, tag="tp", bufs=2)
        nc.tensor.matmul(ps2[:], fc2w_t[:], h1[:], start=True, stop=True)
        outsb = tmp.tile([1, M], F32, tag="outsb")
        nc.scalar.add(outsb[:], ps2[:], fc2b_t[:, 0:1])
        nc.sync.dma_start(d_out.ap(), outsb[:])

    nc.compile()
    return nc


def _prep(inputs):
    """Host preprocessing: returns the 8 per-core in_maps."""
    f32 = np.float32
    fatoms = np.asarray(inputs["fatoms"], f32)
    fbonds = np.asarray(inputs["fbonds"], f32)
    agraph = np.asarray(inputs["agraph"])
    bgraph = np.asarray(inputs["bgraph"])
    pseq = np.asarray(inputs["protein_seq"])
    W_i = np.asarray(inputs["W_i"], f32)
    W_h = np.asarray(inputs["W_h"], f32)
    W_o_w = np.asarray(inputs["W_o_w"], f32)
    W_o_b = np.asarray(inputs["W_o_b"], f32)
    embp = np.asarray(inputs["embed_protein"], f32)

    # protein embeddings, channel-major
    pvT = np.ascontiguousarray(embp[pseq].transpose(0, 2, 1))  # (B, 50, L)

    # adjacency one-hots (counts; contraction-dim-major for lhsT/rhs use)
    ar = np.arange(B)[:, None, None]
    cntB = np.zeros((B, NB, NB), f32)
    np.add.at(cntB, (ar, np.arange(NB)[None, :, None], bgraph), 1.0)
    abt = np.ascontiguousarray(cntB.transpose(0, 2, 1))        # (B, j, i)
    cntA = np.zeros((B, NA, NB), f32)
    np.add.at(cntA, (ar, np.arange(NA)[None, :, None], agraph), 1.0)
    aat = np.ascontiguousarray(cntA.transpose(0, 2, 1))        # (B, j, a)

    fbT = fbonds.transpose(0, 2, 1)                            # (B, 50, 96)
    faT = fatoms.transpose(0, 2, 1)                            # (B, 39, 48)
    cat1 = np.concatenate([faT, np.ones((B, 1, NA), f32)], axis=1)  # (B,40,48)

    def pad_cols(a, n=256):
        out = np.zeros((a.shape[0], n), f32)
        out[:, :a.shape[1]] = a
        return out

    wi = pad_cols(W_i)                                         # (50, 256)
    wh = np.zeros((100, 2, 256), f32)
    wh[:, 0, :200] = W_h[0:100]
    wh[:, 1, :200] = W_h[100:200]
    wo1 = np.zeros((40, 256), f32)
    wo1[:39, :200] = W_o_w[0:39]
    wo1[39, :200] = W_o_b
    wo2 = pad_cols(W_o_w[39:167])
    wo3 = pad_cols(W_o_w[167:239])

    conv_w = [np.asarray(inputs[f"conv{i}_w"], f32) for i in range(3)]
    conv_b = [np.asarray(inputs[f"conv{i}_b"], f32) for i in range(3)]

    import ml_dtypes
    bf16 = ml_dtypes.bfloat16
    e4 = ml_dtypes.float8_e4m3fn

    def q8(a, s):
        # TRN fp8_e4m3 matches OCP e4m3fn bit-for-bit up to |x| <= 240
        return np.clip(np.asarray(a, f32) * s, -240, 240).astype(e4)

    # conv weights (out, in, k) -> fp8 tap-pair packs [in, taps, out]
    cw = [q8(w, SW) for w in conv_w]
    w0dr = np.ascontiguousarray(cw[0][:, :, 0:2].transpose(1, 2, 0))
    w0s = np.ascontiguousarray(cw[0][:, :, 2].transpose(1, 0))
    w1dr = np.ascontiguousarray(cw[1][:, :, 0:4].transpose(1, 2, 0))
    w1s = np.ascontiguousarray(cw[1][:, :, 4].transpose(1, 0))
    w2dra = np.ascontiguousarray(cw[2][0:128, :, 0:6].transpose(1, 2, 0))
    w2sa = np.ascontiguousarray(cw[2][0:128, :, 6].transpose(1, 0))
    w2drb = np.zeros((128, 6, 80), e4)
    w2drb[:, :, 0:72] = cw[2][128:200, :, 0:6].transpose(1, 2, 0)
    w2sb = np.ascontiguousarray(cw[2][128:200, :, 6].transpose(1, 0))

    fcw = [np.asarray(inputs[f"fc{i}_w"], f32) for i in range(3)]
    fcb = [np.asarray(inputs[f"fc{i}_b"], f32) for i in range(3)]

    shared = {
        "wi": wi, "wh": np.ascontiguousarray(wh),
        "wo1": wo1, "wo2": wo2, "wo3": wo3,
        "b0": conv_b[0].reshape(96, 1) * S1,
        "b1": conv_b[1].reshape(128, 1) * S2,
        "b2a": conv_b[2][0:128].reshape(128, 1),
        "b2b": conv_b[2][128:200].reshape(72, 1),
        "fc0a": np.ascontiguousarray(fcw[0][0:128]),
        "fc0b": np.ascontiguousarray(fcw[0][128:200]),
        "fc0c": np.ascontiguousarray(fcw[0][200:328]),
        "fc0d": np.ascontiguousarray(fcw[0][328:400]),
        "fc0ba": fcb[0][0:128].reshape(128, 1),
        "fc0bb": fcb[0][128:200].reshape(72, 1),
        "fc1a": np.ascontiguousarray(fcw[1][0:128]),
        "fc1b": np.ascontiguousarray(fcw[1][128:200]),
        "fc1bias": fcb[1].reshape(100, 1),
        "fc2w": np.ascontiguousarray(fcw[2]),
        "fc2b": fcb[2].reshape(1, 1),
        "ones48": np.ones((48, 1), f32),
    }
    shared = {k: np.ascontiguousarray(v, f32) for k, v in shared.items()}
    shared.update({"w0dr": w0dr, "w0s": w0s, "w1dr": w1dr, "w1s": w1s,
                   "w2dra": w2dra, "w2sa": w2sa, "w2drb": w2drb,
                   "w2sb": w2sb})

    # protein activations: fp8 two-plane layout with conv pads baked in
    # plane0[:, PAD+q] = x[q]; plane1[:, PAD-1+q] = x[q]
    pvq = q8(pvT, S0)                                          # (B, 50, L)
    pvt_pad = np.zeros((B, 50, 2, SEG), e4)
    pvt_pad[:, :, 0, PAD:PAD + L] = pvq
    pvt_pad[:, :, 1, PAD - 1:PAD - 1 + L] = pvq

    in_maps = []
    for c in range(NCORES):
        lo = c * M
        im = dict(shared)
        for g in range(M):
            im[f"pvt{g}"] = np.ascontiguousarray(pvt_pad[lo + g])  # (50, SEG)
        im["fbt"] = np.ascontiguousarray(fbT[lo:lo + M].transpose(1, 0, 2))
        im["cat1"] = np.ascontiguousarray(cat1[lo:lo + M].transpose(1, 0, 2))
        im["abt"] = np.ascontiguousarray(
            abt[lo:lo + M].transpose(1, 0, 2)).astype(bf16)
        im["aat"] = np.ascontiguousarray(
            aat[lo:lo + M].transpose(1, 0, 2)).astype(bf16)
        in_maps.append(im)
    return in_maps


def get_nc():
    if "nc" not in _CACHE:
        _CACHE["nc"] = _build_nc()
    return _CACHE["nc"]


def kernel(**inputs) -> np.ndarray:
    nc = get_nc()
    in_maps = _prep(inputs)
    res = run_bass_kernel_spmd(nc, in_maps, core_ids=list(range(NCORES)))
    outs = [res.results[c]["out"].reshape(M, 1) for c in range(NCORES)]
    return np.concatenate(outs, axis=0).astype(np.float32)



# revision 20
# speedup vs baseline: 1.3622x; 1.3622x over previous
"""CPI-MPNN (molecule MPNN + protein CNN + FC head) Trainium2 kernel.

Self-contained: hardcodes all shapes. Shards the batch (128) across 8
NeuronCores (16 samples each), replicates the small weights.

Strategy:
  - Host (numpy): protein embedding gather (-> channel-major, fp8 e4m3
    with power-of-2 scaling, conv pads + the tap-shifted second plane
    baked in), bond/atom graph one-hot adjacency matrices (gather+sum
    == matmul), weight transposes, fp8 bias-offset folding.
  - Conv tower in fp8 e4m3 with DoubleRow matmuls (2 taps per pass ->
    ~half the PE streaming cycles; end-to-end quantization error ~1e-3
    vs the 2e-2 gate). Conv1d = per-tap-pair matmuls accumulated in
    PSUM over a zero-padded two-plane activation layout (plane1 =
    plane0 shifted one position, so one DoubleRow pass covers taps
    (d, d+1)). The relu+scale+bias epilogue is algebraically folded to
    2 ALU ops -- stored x~ = S*(relu(conv+b) - beta) = S*max(psum*k,
    -beta) -- so it runs as one tensor_scalar on DVE (x1) / Pool (x2)
    instead of the half-rate fp8-output scalar ACT; the per-channel
    offset beta is position-independent, so its effect on the next conv
    is a host-precomputed bias correction. Pads hold -S*beta.
  - Maxpool runs on raw PSUM before bias+relu (monotone + positive
    scale), split DVE/Pool.
  - MPNN in bf16 (error ~5e-3 total): aggregation emitted twice per
    iteration with msg as the stationary operand so nei lands directly
    transposed (kills the per-iteration PE transposes), W_h/W_i/W_o and
    the FC head in bf16 at N=200.
"""

import numpy as np
from contextlib import ExitStack

import concourse.bass as bass
import concourse.tile as tile
from concourse import bacc, mybir
from concourse.bass_utils import run_bass_kernel_spmd

F32 = mybir.dt.float32
F32R = mybir.dt.float32r
BF16 = mybir.dt.bfloat16
FP8 = mybir.dt.float8e4
DR = mybir.MatmulPerfMode.DoubleRow
AF = mybir.ActivationFunctionType
ALU = mybir.AluOpType

# model dims
H = 200
B, NA, NB = 128, 48, 96
L, VOCAB = 1000, 26

NCORES = 8
M = B // NCORES          # molecules per core (16)
SEG = 1008               # 3 + 1000 + 5 padded segment (16B-aligned plane)
PAD = 3
NCH = 500                # conv free-dim chunk (2 per sample)

# fp8 power-of-2 scales (exactly cancelled in the psum epilogues)
S0 = 128.0               # protein embedding activations
SW = 128.0               # conv weights
S1 = 256.0               # x1 activations
S2 = 256.0               # x2 activations
ACT0_SCALE = S1 / (S0 * SW)    # 2^-6
ACT1_SCALE = S2 / (S1 * SW)    # 2^-7
FIN_SCALE = 1.0 / (S2 * SW)    # 2^-15

_CACHE = {}


def _build_nc():
    nc = bacc.Bacc("TRN2", target_bir_lowering=False, debug=False)

    # ---- DRAM inputs (per core) ----
    # protein activations go over the wire in fp8 (scaled by S0) as two
    # planes: plane1 = plane0 shifted one position, so a DoubleRow AP
    # over both planes covers the tap pair (d, d+1).
    d_pvt = [nc.dram_tensor(f"pvt{g}", [50, 2, SEG], FP8, kind="ExternalInput")
             for g in range(M)]
    d_fbt = nc.dram_tensor("fbt", [50, M, 96], BF16, kind="ExternalInput")
    d_cat1 = nc.dram_tensor("cat1", [40, M, 48], BF16, kind="ExternalInput")
    # adjacency counts are small integers: exact in bf16
    d_abt = nc.dram_tensor("abt", [96, M, 96], BF16, kind="ExternalInput")
    d_aat = nc.dram_tensor("aat", [96, M, 48], BF16, kind="ExternalInput")

    d_wi = nc.dram_tensor("wi", [50, 200], BF16, kind="ExternalInput")
    d_wh = nc.dram_tensor("wh", [100, 2, 200], BF16, kind="ExternalInput")
    d_wo1 = nc.dram_tensor("wo1", [40, 200], BF16, kind="ExternalInput")
    d_wo2 = nc.dram_tensor("wo2", [128, 200], BF16, kind="ExternalInput")
    d_wo3 = nc.dram_tensor("wo3", [72, 200], BF16, kind="ExternalInput")
    # conv weights, fp8: DoubleRow tap-pair packs + the odd single tap
    d_w0dr = nc.dram_tensor("w0dr", [50, 2, 96], FP8, kind="ExternalInput")
    d_w0s = nc.dram_tensor("w0s", [50, 96], FP8, kind="ExternalInput")
    d_w1dr = nc.dram_tensor("w1dr", [96, 4, 128], FP8, kind="ExternalInput")
    d_w1s = nc.dram_tensor("w1s", [96, 128], FP8, kind="ExternalInput")
    d_w2dra = nc.dram_tensor("w2dra", [128, 6, 128], FP8, kind="ExternalInput")
    d_w2sa = nc.dram_tensor("w2sa", [128, 128], FP8, kind="ExternalInput")
    # 72-col group padded to 80 so the DoubleRow plane step is 16B-aligned
    d_w2drb = nc.dram_tensor("w2drb", [128, 6, 80], FP8, kind="ExternalInput")
    d_w2sb = nc.dram_tensor("w2sb", [128, 72], FP8, kind="ExternalInput")
    # epilogue constants: nb = -(Sin*SW)*beta (psum-domain max operand),
    # pb = -Sout*beta (fp8 pad value, f32 on the wire)
    d_nb0 = nc.dram_tensor("nb0", [96, 1], F32, kind="ExternalInput")
    d_pb1 = nc.dram_tensor("pb1", [96, 1], F32, kind="ExternalInput")
    d_b1c = nc.dram_tensor("b1c", [128, 1], F32, kind="ExternalInput")
    d_b2a = nc.dram_tensor("b2a", [128, 1], F32, kind="ExternalInput")
    d_b2b = nc.dram_tensor("b2b", [72, 1], F32, kind="ExternalInput")
    d_fc0 = [nc.dram_tensor(f"fc0{k}", [dim, 200], BF16, kind="ExternalInput")
             for k, dim in (("a", 128), ("b", 72), ("c", 128), ("d", 72))]
    d_fc0ba = nc.dram_tensor("fc0ba", [128, 1], F32, kind="ExternalInput")
    d_fc0bb = nc.dram_tensor("fc0bb", [72, 1], F32, kind="ExternalInput")
    d_fc1a = nc.dram_tensor("fc1a", [128, 100], BF16, kind="ExternalInput")
    d_fc1b = nc.dram_tensor("fc1b", [72, 100], BF16, kind="ExternalInput")
    d_fc1bias = nc.dram_tensor("fc1bias", [100, 1], F32, kind="ExternalInput")
    d_fc2w = nc.dram_tensor("fc2w", [100, 1], BF16, kind="ExternalInput")
    d_fc2b = nc.dram_tensor("fc2b", [1, 1], F32, kind="ExternalInput")
    d_ones = nc.dram_tensor("ones48", [48, 1], BF16, kind="ExternalInput")

    d_out = nc.dram_tensor("out", [1, M], F32, kind="ExternalOutput")

    with tile.TileContext(nc) as tc, ExitStack() as ctx:
        cst = ctx.enter_context(tc.tile_pool(name="cst", bufs=1))
        sbs = ctx.enter_context(tc.tile_pool(name="sbs", bufs=1))   # static per-mol state
        tmp = ctx.enter_context(tc.tile_pool(name="tmp", bufs=1))
        xp = ctx.enter_context(tc.tile_pool(name="xp", bufs=1))
        pp = ctx.enter_context(tc.tile_pool(name="pp", bufs=1, space="PSUM"))

        # ---- load constants ----
        # DMA issue order matters: the critical path at kernel start is
        # (wi, fbt, abt) for the MPNN and (w0, pvt0) for the conv.
        def const_tile(dram, shape, dtype=BF16, name=None, eng=None):
            t = cst.tile(shape, dtype, tag=name or dram.name)
            (eng or nc.sync).dma_start(t[:], dram.ap())
            return t

        # MPNN inputs arrive in 4 molecule-groups so mol 0 isn't gated on
        # the whole batch. Group g covers mols 4g..4g+3.
        GM = 4
        fbt_g, abt_g, aat_g, cat1_g = {}, {}, {}, {}

        def fbt_dma(g):
            t = cst.tile([50, GM * 96], BF16, tag=f"fbt{g}")
            nc.sync.dma_start(t[:].rearrange("p (m i) -> p m i", i=96),
                              d_fbt.ap()[:, GM * g:GM * (g + 1), :])
            fbt_g[g] = t

        def abt_dma(g):
            t = cst.tile([96, GM * 96], BF16, tag=f"abt{g}")
            nc.sync.dma_start(t[:].rearrange("p (m i) -> p m i", i=96),
                              d_abt.ap()[:, GM * g:GM * (g + 1), :])
            abt_g[g] = t

        def aat_cat_dma(g, eng):
            t = cst.tile([96, GM * 48], BF16, tag=f"aat{g}")
            eng.dma_start(t[:].rearrange("p (m i) -> p m i", i=48),
                          d_aat.ap()[:, GM * g:GM * (g + 1), :])
            aat_g[g] = t
            t = cst.tile([40, GM * 48], BF16, tag=f"cat1{g}")
            eng.dma_start(t[:].rearrange("p (m i) -> p m i", i=48),
                          d_cat1.ap()[:, GM * g:GM * (g + 1), :])
            cat1_g[g] = t

        # SP queue: mol-group-0 first, then wh / conv2 weights, then
        # later groups.
        wi_t = const_tile(d_wi, [50, 200])
        fbt_dma(0)
        abt_dma(0)
        wh_t = cst.tile([100, 2 * 200], BF16, tag="wh")
        nc.sync.dma_start(wh_t[:].rearrange("p (c n) -> p c n", n=200),
                          d_wh.ap())
        w2dra_t = cst.tile([128, 6 * 128], FP8, tag="w2dra")
        nc.sync.dma_start(w2dra_t[:].rearrange("p (t o) -> p t o", o=128),
                          d_w2dra.ap())
        w2sa_t = const_tile(d_w2sa, [128, 128], FP8)
        w2drb_t = cst.tile([128, 6 * 80], FP8, tag="w2drb")
        nc.sync.dma_start(w2drb_t[:].rearrange("p (t o) -> p t o", o=80),
                          d_w2drb.ap())
        w2sb_t = const_tile(d_w2sb, [128, 72], FP8)
        aat_cat_dma(0, nc.sync)
        wo1_t = const_tile(d_wo1, [40, 200], eng=nc.sync)
        wo2_t = const_tile(d_wo2, [128, 200], eng=nc.sync)
        wo3_t = const_tile(d_wo3, [72, 200], eng=nc.sync)
        ones_t = const_tile(d_ones, [48, 1], eng=nc.sync)
        b2a_t = const_tile(d_b2a, [128, 1], F32, eng=nc.sync)
        b2b_t = const_tile(d_b2b, [72, 1], F32, eng=nc.sync)
        b1c_t = const_tile(d_b1c, [128, 1], F32, eng=nc.sync)
        fbt_dma(1)
        abt_dma(1)
        aat_cat_dma(1, nc.sync)
        fbt_dma(2)
        abt_dma(2)
        aat_cat_dma(2, nc.sync)
        fbt_dma(3)
        abt_dma(3)
        aat_cat_dma(3, nc.sync)
        fc0_t = [const_tile(d, [dim, 200], eng=nc.sync) for d, dim in
                 zip(d_fc0, (128, 72, 128, 72))]
        fc0ba_t = const_tile(d_fc0ba, [128, 1], F32, eng=nc.sync)
        fc0bb_t = const_tile(d_fc0bb, [72, 1], F32, eng=nc.sync)
        fc1a_t = const_tile(d_fc1a, [128, 100], eng=nc.sync)
        fc1b_t = const_tile(d_fc1b, [72, 100], eng=nc.sync)
        fc1bias_t = const_tile(d_fc1bias, [100, 1], F32, eng=nc.sync)
        fc2w_t = const_tile(d_fc2w, [100, 1], eng=nc.sync)
        fc2b_t = const_tile(d_fc2b, [1, 1], F32, eng=nc.sync)

        # ACT queue: conv weights + per-sample x0 buffers
        w0dr_t = cst.tile([50, 2 * 96], FP8, tag="w0dr")
        nc.scalar.dma_start(w0dr_t[:].rearrange("p (t o) -> p t o", o=96),
                            d_w0dr.ap())
        w0s_t = const_tile(d_w0s, [50, 96], FP8, eng=nc.scalar)
        nb0_t = const_tile(d_nb0, [96, 1], F32, eng=nc.scalar)
        pb1_t = const_tile(d_pb1, [96, 1], F32, eng=nc.scalar)
        x0_bufs = []

        def pvt_dma(s):
            t = xp.tile([50, 2 * SEG], FP8, tag=f"x0s{s}")
            nc.scalar.dma_start(t[:].rearrange("p (k c) -> p k c", k=2),
                                d_pvt[s].ap())
            x0_bufs.append(t)

        w1dr_t = cst.tile([96, 4 * 128], FP8, tag="w1dr")
        nc.gpsimd.dma_start(w1dr_t[:].rearrange("p (t o) -> p t o", o=128),
                            d_w1dr.ap())
        w1s_t = const_tile(d_w1s, [96, 128], FP8, eng=nc.gpsimd)
        pvt_dma(0)
        pvt_dma(1)
        # remaining pvt loads are issued inside emit_sample_front so the
        # ACT sequencer isn't blocked issuing DMAs ahead of its compute.

        zt = cst.tile([128, 8], F32, tag="zt")
        nc.gpsimd.memset(zt[:], 0.0)

        # static outputs of the two towers, feature-major [feat, M]
        embT1 = sbs.tile([128, M], BF16, tag="embT1")
        embT2 = sbs.tile([72, M], BF16, tag="embT2")
        prT1p = sbs.tile([128, M], F32, tag="prT1p")
        prT2p = sbs.tile([72, M], F32, tag="prT2p")

        # ================= per-molecule MPNN (staged, bf16) ============
        mol_state = {}

        def emit_binput(m):
            g, r = m // GM, m % GM
            fb_m = fbt_g[g][:, r * 96:(r + 1) * 96]
            psA = pp.tile([96, 200], F32, tag="mp", bufs=3)
            nc.tensor.matmul(psA[:], fb_m, wi_t[:], start=True, stop=True)
            binp = sbs.tile([96, 200], F32, tag=f"binp{m}")
            nc.scalar.copy(binp[:], psA[:])
            msg = sbs.tile([96, 200], BF16, tag=f"msg{m}")
            nc.scalar.activation(msg[:], psA[:], AF.Relu)
            mol_state[m] = (binp, msg)

        def emit_iter_pre(m):
            # nei, directly transposed: psT = msg^T-slices @ A
            g, r = m // GM, m % GM
            ab_m = abt_g[g][:, r * 96:(r + 1) * 96]
            binp, msg = mol_state[m]
            psTa = pp.tile([100, 96], F32, tag="tp", bufs=2)
            nc.tensor.matmul(psTa[:], msg[:, 0:100], ab_m, start=True, stop=True)
            nTa = tmp.tile([100, 96], BF16, tag="nTa", bufs=6)
            nc.scalar.copy(nTa[:], psTa[:])
            psTb = pp.tile([100, 96], F32, tag="tp", bufs=2)
            nc.tensor.matmul(psTb[:], msg[:, 100:200], ab_m, start=True, stop=True)
            nTb = tmp.tile([100, 96], BF16, tag="nTb", bufs=6)
            nc.scalar.copy(nTb[:], psTb[:])
            mol_state[m] = (binp, msg, nTa, nTb)

        def emit_iter_post(m):
            binp, msg, nTa, nTb = mol_state[m]
            psH = pp.tile([96, 200], F32, tag="mp", bufs=3)
            nc.tensor.matmul(psH[:], nTa[:], wh_t[:, 0:200],
                             start=True, stop=False)
            nc.tensor.matmul(psH[:], nTb[:], wh_t[:, 200:400],
                             start=False, stop=True)
            tm = tmp.tile([96, 200], F32, tag="mtmp", bufs=3)
            nc.vector.tensor_add(tm[:], psH[:], binp[:])
            nc.gpsimd.tensor_scalar(msg[:], tm[:], 0.0, None, op0=ALU.max)
            mol_state[m] = (binp, msg)

        def emit_atom(m):
            g, r = m // GM, m % GM
            aa_m = aat_g[g][:, r * 48:(r + 1) * 48]
            c1_m = cat1_g[g][:, r * 48:(r + 1) * 48]
            binp, msg = mol_state[m]
            psT1 = pp.tile([128, 48], F32, tag="tp", bufs=2)
            nc.tensor.matmul(psT1[:], msg[:, 0:128], aa_m, start=True, stop=True)
            nat1 = tmp.tile([128, 48], BF16, tag="nat1", bufs=3)
            nc.scalar.copy(nat1[:], psT1[:])
            psT2 = pp.tile([72, 48], F32, tag="tp", bufs=2)
            nc.tensor.matmul(psT2[:], msg[:, 128:200], aa_m, start=True, stop=True)
            nat2 = tmp.tile([72, 48], BF16, tag="nat2", bufs=3)
            nc.scalar.copy(nat2[:], psT2[:])

            psAH = pp.tile([48, 200], F32, tag="mp", bufs=3)
            nc.tensor.matmul(psAH[:], c1_m, wo1_t[:], start=True, stop=False)
            nc.tensor.matmul(psAH[:], nat1[:], wo2_t[:], start=False, stop=False)
            nc.tensor.matmul(psAH[:], nat2[:], wo3_t[:], start=False, stop=True)
            reluh = tmp.tile([48, 200], BF16, tag="reluh", bufs=3)
            nc.scalar.activation(reluh[:], psAH[:], AF.Relu)

            psE1 = pp.tile([128, 1], F32, tag="tp", bufs=2)
            nc.tensor.matmul(psE1[:], reluh[:, 0:128], ones_t[:],
                             start=True, stop=True)
            nc.scalar.mul(embT1[:, m:m + 1], psE1[:], 1.0 / 48)
            psE2 = pp.tile([72, 1], F32, tag="tp", bufs=2)
            nc.tensor.matmul(psE2[:], reluh[:, 128:200], ones_t[:],
                             start=True, stop=True)
            nc.scalar.mul(embT2[:, m:m + 1], psE2[:], 1.0 / 48)

        # ================= per-sample protein conv tower =================
        # x buffers hold two fp8 planes; plane1 = plane0 shifted one
        # position left, so a DoubleRow AP with base offset o covers the
        # tap pair (x[p+o-PAD], x[p+o-PAD+1]). Data lives at plane0 cols
        # [PAD, PAD+1000); plane1 cols [PAD-1, PAD+999). Pads carry the
        # folded offset value -S*beta (pb) instead of zero.
        sample_state = {}

        def pads(xflat, rows, pb, eng):
            # flat strips: plane0 head, plane0 tail + plane1 head, plane1 tail
            for a, b in ((0, PAD), (SEG - 5, SEG + PAD - 1),
                         (2 * SEG - 6, 2 * SEG)):
                if pb is None:
                    eng.memset(xflat[:, a:b], 0.0)
                else:
                    eng.tensor_scalar(xflat[:, a:b], zt[0:rows, 0:b - a],
                                      pb[:], None, op0=ALU.add)

        def emit_sample_front(s):
            if s + 2 < M:
                pvt_dma(s + 2)
            x03 = x0_bufs[s][:].rearrange("p (k c) -> p k c", k=2)
            w0dr3 = w0dr_t[:].rearrange("p (k o) -> p k o", o=96)
            x1 = xp.tile([96, 2 * SEG], FP8, tag="x1", bufs=3)
            x13 = x1[:].rearrange("p (k c) -> p k c", k=2)
            pads(x1[:], 96, pb1_t, nc.vector)
            for c in range(2):
                o = c * NCH
                ps = pp.tile([96, NCH], F32, tag="cv", bufs=3)
                nc.tensor.matmul(ps[:], w0dr3, x03[:, :, o + 2:o + 2 + NCH],
                                 start=True, stop=False, perf_mode=DR)
                nc.tensor.matmul(ps[:], w0s_t[:], x03[:, 0, o + 4:o + 4 + NCH],
                                 start=False, stop=True)
                # x~1 = S1*max(psum,-2^14*b0)/2^14  (= S1*(relu(conv+b0)-b0))
                nc.vector.tensor_scalar(x13[:, 0, PAD + o:PAD + o + NCH],
                                        ps[:], nb0_t[:], ACT0_SCALE,
                                        op0=ALU.max, op1=ALU.mult)
                nc.vector.tensor_scalar(x13[:, 1, PAD - 1 + o:PAD - 1 + o + NCH],
                                        ps[:], nb0_t[:], ACT0_SCALE,
                                        op0=ALU.max, op1=ALU.mult)

            w1dr3 = w1dr_t[:].rearrange("p (t o) -> p t o", o=128)
            x2 = xp.tile([128, 2 * SEG], FP8, tag="x2", bufs=4)
            x23 = x2[:].rearrange("p (k c) -> p k c", k=2)
            pads(x2[:], 128, None, nc.gpsimd)
            for c in range(2):
                o = c * NCH
                ps = pp.tile([128, NCH], F32, tag="cv", bufs=3)
                nc.tensor.matmul(ps[:], w1dr3[:, 0:2, :],
                                 x13[:, :, o + 1:o + 1 + NCH],
                                 start=True, stop=False, perf_mode=DR)
                nc.tensor.matmul(ps[:], w1dr3[:, 2:4, :],
                                 x13[:, :, o + 3:o + 3 + NCH],
                                 start=False, stop=False, perf_mode=DR)
                nc.tensor.matmul(ps[:], w1s_t[:], x13[:, 0, o + 5:o + 5 + NCH],
                                 start=False, stop=True)
                nc.scalar.activation(x23[:, 0, PAD + o:PAD + o + NCH],
                                     ps[:], AF.Relu, bias=b1c_t[:],
                                     scale=ACT1_SCALE)
                nc.scalar.activation(x23[:, 1, PAD - 1 + o:PAD - 1 + o + NCH],
                                     ps[:], AF.Relu, bias=b1c_t[:],
                                     scale=ACT1_SCALE)
            sample_state[s] = x2

        def emit_sample_back(s):
            x2 = sample_state.pop(s)
            x23 = x2[:].rearrange("p (k c) -> p k c", k=2)
            w2dra3 = w2dra_t[:].rearrange("p (t o) -> p t o", o=128)
            w2drb3 = w2drb_t[:].rearrange("p (t o) -> p t o", o=80)
            mxA = tmp.tile([128, 2], F32, tag="mxA", bufs=3)
            mxB = tmp.tile([72, 2], F32, tag="mxB", bufs=3)
            for c in range(2):
                o = c * NCH
                psA = pp.tile([128, NCH], F32, tag="cv", bufs=3)
                for p in range(3):
                    nc.tensor.matmul(psA[:], w2dra3[:, 2 * p:2 * p + 2, :],
                                     x23[:, :, o + 2 * p:o + 2 * p + NCH],
                                     start=(p == 0), stop=False, perf_mode=DR)
                nc.tensor.matmul(psA[:], w2sa_t[:],
                                 x23[:, 0, o + 6:o + 6 + NCH],
                                 start=False, stop=True)
                nc.vector.reduce_max(mxA[:, c:c + 1], psA[:],
                                     axis=mybir.AxisListType.X)
                psB = pp.tile([72, NCH], F32, tag="cv", bufs=3)
                for p in range(3):
                    nc.tensor.matmul(psB[:], w2drb3[:, 2 * p:2 * p + 2, 0:72],
                                     x23[:, :, o + 2 * p:o + 2 * p + NCH],
                                     start=(p == 0), stop=False, perf_mode=DR)
                nc.tensor.matmul(psB[:], w2sb_t[:],
                                 x23[:, 0, o + 6:o + 6 + NCH],
                                 start=False, stop=True)
                nc.vector.reduce_max(mxB[:, c:c + 1], psB[:],
                                     axis=mybir.AxisListType.X)
            nc.vector.reduce_max(prT1p[:, s:s + 1], mxA[:],
                                 axis=mybir.AxisListType.X)
            nc.vector.reduce_max(prT2p[:, s:s + 1], mxB[:],
                                 axis=mybir.AxisListType.X)

        # Stage-interleaved emission: conv stages slot between MPNN
        # stages so the in-order PE stream always has DMA-ready filler.
        for g in range(4):
            mols = [GM * g + r for r in range(GM)]
            for m in mols:
                emit_binput(m)
            for m in mols:
                emit_iter_pre(m)
            emit_sample_front(4 * g + 0)
            emit_sample_front(4 * g + 1)
            for m in mols:
                emit_iter_post(m)
            for m in mols:
                emit_iter_pre(m)
            emit_sample_back(4 * g + 0)
            for m in mols:
                emit_iter_post(m)
            emit_sample_front(4 * g + 2)
            emit_sample_back(4 * g + 1)
            for m in mols:
                emit_atom(m)
            emit_sample_front(4 * g + 3)
            emit_sample_back(4 * g + 2)
            emit_sample_back(4 * g + 3)

        # maxpool -> descale -> bias(+C2) -> relu (monotone, pool-first
        # exact); bf16 out for the bf16 FC head
        prT1 = sbs.tile([128, M], BF16, tag="prT1")
        nc.scalar.activation(prT1[:], prT1p[:], AF.Relu, bias=b2a_t[:],
                             scale=FIN_SCALE)
        prT2 = sbs.tile([72, M], BF16, tag="prT2")
        nc.scalar.activation(prT2[:], prT2p[:], AF.Relu, bias=b2b_t[:],
                             scale=FIN_SCALE)

        # ================= FC head (bf16) =================
        rhs4 = (embT1, embT2, prT1, prT2)
        ps0a = pp.tile([128, M], F32, tag="tp", bufs=2)
        for k in range(4):
            nc.tensor.matmul(ps0a[:], fc0_t[k][:, 0:128], rhs4[k][:],
                             start=(k == 0), stop=(k == 3))
        h0a = tmp.tile([128, M], BF16, tag="h0a")
        nc.scalar.activation(h0a[:], ps0a[:], AF.Relu, bias=fc0ba_t[:])
        ps0b = pp.tile([72, M], F32, tag="tp", bufs=2)
        for k in range(4):
            nc.tensor.matmul(ps0b[:], fc0_t[k][:, 128:200], rhs4[k][:],
                             start=(k == 0), stop=(k == 3))
        h0b = tmp.tile([72, M], BF16, tag="h0b")
        nc.scalar.activation(h0b[:], ps0b[:], AF.Relu, bias=fc0bb_t[:])

        ps1 = pp.tile([100, M], F32, tag="tp", bufs=2)
        nc.tensor.matmul(ps1[:], fc1a_t[:], h0a[:], start=True, stop=False)
        nc.tensor.matmul(ps1[:], fc1b_t[:], h0b[:], start=False, stop=True)
        h1 = tmp.tile([100, M], BF16, tag="h1")
        nc.scalar.activation(h1[:], ps1[:], AF.Relu, bias=fc1bias_t[:])

        ps2 = pp.tile([1, M], F32, tag="tp", bufs=2)
        nc.tensor.matmul(ps2[:], fc2w_t[:], h1[:], start=True, stop=True)
        outsb = tmp.tile([1, M], F32, tag="outsb")
        nc.scalar.add(outsb[:], ps2[:], fc2b_t[:, 0:1])
        nc.sync.dma_start(d_out.ap(), outsb[:])

    nc.compile()
    return nc


def _prep(inputs):
    """Host preprocessing: returns the 8 per-core in_maps."""
    f32 = np.float32
    fatoms = np.asarray(inputs["fatoms"], f32)
    fbonds = np.asarray(inputs["fbonds"], f32)
    agraph = np.asarray(inputs["agraph"])
    bgraph = np.asarray(inputs["bgraph"])
    pseq = np.asarray(inputs["protein_seq"])
    W_i = np.asarray(inputs["W_i"], f32)
    W_h = np.asarray(inputs["W_h"], f32)
    W_o_w = np.asarray(inputs["W_o_w"], f32)
    W_o_b = np.asarray(inputs["W_o_b"], f32)
    embp = np.asarray(inputs["embed_protein"], f32)

    import ml_dtypes
    bf16 = ml_dtypes.bfloat16
    e4 = ml_dtypes.float8_e4m3fn

    def q8(a, s):
        # TRN fp8_e4m3 matches OCP e4m3fn bit-for-bit up to |x| <= 240
        return np.clip(np.asarray(a, f32) * s, -240, 240).astype(e4)

    # protein embeddings, channel-major
    pvT = np.ascontiguousarray(embp[pseq].transpose(0, 2, 1))  # (B, 50, L)

    # adjacency one-hots (counts; contraction-dim-major for lhsT/rhs use)
    ar = np.arange(B)[:, None, None]
    cntB = np.zeros((B, NB, NB), f32)
    np.add.at(cntB, (ar, np.arange(NB)[None, :, None], bgraph), 1.0)
    abt = np.ascontiguousarray(cntB.transpose(0, 2, 1))        # (B, j, i)
    cntA = np.zeros((B, NA, NB), f32)
    np.add.at(cntA, (ar, np.arange(NA)[None, :, None], agraph), 1.0)
    aat = np.ascontiguousarray(cntA.transpose(0, 2, 1))        # (B, j, a)

    fbT = fbonds.transpose(0, 2, 1)                            # (B, 50, 96)
    faT = fatoms.transpose(0, 2, 1)                            # (B, 39, 48)
    cat1 = np.concatenate([faT, np.ones((B, 1, NA), f32)], axis=1)  # (B,40,48)

    wh = np.zeros((100, 2, 200), f32)
    wh[:, 0, :] = W_h[0:100]
    wh[:, 1, :] = W_h[100:200]
    wo1 = np.zeros((40, 200), f32)
    wo1[:39, :] = W_o_w[0:39]
    wo1[39, :] = W_o_b

    conv_w = [np.asarray(inputs[f"conv{i}_w"], f32) for i in range(3)]
    conv_b = [np.asarray(inputs[f"conv{i}_b"], f32) for i in range(3)]

    # conv weights (out, in, k) -> fp8 tap-pair packs [in, taps, out]
    cw = [q8(w, SW) for w in conv_w]
    w0dr = np.ascontiguousarray(cw[0][:, :, 0:2].transpose(1, 2, 0))
    w0s = np.ascontiguousarray(cw[0][:, :, 2].transpose(1, 0))
    w1dr = np.ascontiguousarray(cw[1][:, :, 0:4].transpose(1, 2, 0))
    w1s = np.ascontiguousarray(cw[1][:, :, 4].transpose(1, 0))
    w2dra = np.ascontiguousarray(cw[2][0:128, :, 0:6].transpose(1, 2, 0))
    w2sa = np.ascontiguousarray(cw[2][0:128, :, 6].transpose(1, 0))
    w2drb = np.zeros((128, 6, 80), e4)
    w2drb[:, :, 0:72] = cw[2][128:200, :, 0:6].transpose(1, 2, 0)
    w2sb = np.ascontiguousarray(cw[2][128:200, :, 6].transpose(1, 0))

    # fp8 epilogue offset folding: stored x~ = S*(relu(conv+b) - beta).
    # beta1 = b0; the position-independent offset propagates into the
    # next layer's bias: C[o] = sum_cin,t (wq[o,cin,t]/SW) * beta[cin].
    w1q = cw[1].astype(f32) / SW                               # (128, 96, 5)
    beta1 = conv_b[0]                                          # (96,)
    C1 = np.einsum("ock,c->o", w1q, beta1)                     # (128,)
    b1c = S2 * (conv_b[1] + C1)                                # (128,)

    fcw = [np.asarray(inputs[f"fc{i}_w"], f32) for i in range(3)]
    fcb = [np.asarray(inputs[f"fc{i}_b"], f32) for i in range(3)]

    shared_f32 = {
        "nb0": (-(S0 * SW) * beta1).reshape(96, 1),
        "pb1": (-S1 * beta1).reshape(96, 1),
        "b1c": b1c.reshape(128, 1),
        "b2a": conv_b[2][0:128].reshape(128, 1),
        "b2b": conv_b[2][128:200].reshape(72, 1),
        "fc0ba": fcb[0][0:128].reshape(128, 1),
        "fc0bb": fcb[0][128:200].reshape(72, 1),
        "fc1bias": fcb[1].reshape(100, 1),
        "fc2b": fcb[2].reshape(1, 1),
    }
    shared_bf16 = {
        "wi": W_i, "wh": wh,
        "wo1": wo1, "wo2": W_o_w[39:167], "wo3": W_o_w[167:239],
        "fc0a": fcw[0][0:128], "fc0b": fcw[0][128:200],
        "fc0c": fcw[0][200:328], "fc0d": fcw[0][328:400],
        "fc1a": fcw[1][0:128], "fc1b": fcw[1][128:200],
        "fc2w": fcw[2], "ones48": np.ones((48, 1), f32),
    }
    shared = {k: np.ascontiguousarray(v, f32) for k, v in shared_f32.items()}
    shared.update({k: np.ascontiguousarray(v, f32).astype(bf16)
                   for k, v in shared_bf16.items()})
    shared.update({"w0dr": w0dr, "w0s": w0s, "w1dr": w1dr, "w1s": w1s,
                   "w2dra": w2dra, "w2sa": w2sa, "w2drb": w2drb,
                   "w2sb": w2sb})

    # protein activations: fp8 two-plane layout with conv pads baked in
    # plane0[:, PAD+q] = x[q]; plane1[:, PAD-1+q] = x[q] (x0 pads stay 0)
    pvq = q8(pvT, S0)                                          # (B, 50, L)
    pvt_pad = np.zeros((B, 50, 2, SEG), e4)
    pvt_pad[:, :, 0, PAD:PAD + L] = pvq
    pvt_pad[:, :, 1, PAD - 1:PAD - 1 + L] = pvq

    in_maps = []
    for c in range(NCORES):
        lo = c * M
        im = dict(shared)
        for g in range(M):
            im[f"pvt{g}"] = np.ascontiguousarray(pvt_pad[lo + g])
        im["fbt"] = np.ascontiguousarray(
            fbT[lo:lo + M].transpose(1, 0, 2)).astype(bf16)
        im["cat1"] = np.ascontiguousarray(
            cat1[lo:lo + M].transpose(1, 0, 2)).astype(bf16)
        im["abt"] = np.ascontiguousarray(
            abt[lo:lo + M].transpose(1, 0, 2)).astype(bf16)
        im["aat"] = np.ascontiguousarray(
            aat[lo:lo + M].transpose(1, 0, 2)).astype(bf16)
        in_maps.append(im)
    return in_maps


def get_nc():
    if "nc" not in _CACHE:
        _CACHE["nc"] = _build_nc()
    return _CACHE["nc"]


def kernel(**inputs) -> np.ndarray:
    nc = get_nc()
    in_maps = _prep(inputs)
    res = run_bass_kernel_spmd(nc, in_maps, core_ids=list(range(NCORES)))
    outs = [res.results[c]["out"].reshape(M, 1) for c in range(NCORES)]
    return np.concatenate(outs, axis=0).astype(np.float32)


# revision 23
# speedup vs baseline: 1.6050x; 1.1782x over previous
"""CPI-MPNN (molecule MPNN + protein CNN + FC head) Trainium2 kernel.

Self-contained: hardcodes all shapes. Shards the batch (128) across 8
NeuronCores (16 samples each), replicates the small weights.

Strategy:
  - Host (numpy): protein embedding gather (-> channel-major, fp8 e4m3
    with power-of-2 scaling, conv pads + the tap-shifted second plane
    baked in), bond/atom graph one-hot adjacency matrices (gather+sum
    == matmul), weight transposes.
  - Conv tower in fp8 e4m3 with DoubleRow matmuls (2 taps per pass ->
    ~half the PE streaming cycles; quantization error ~1e-3 vs the
    2e-2 gate). Conv1d = per-tap-pair matmuls accumulated in PSUM over
    a zero-padded two-plane activation layout (plane1 = plane0 shifted
    one position, so one DoubleRow pass covers taps (d, d+1)). Scales
    are chosen so psum arrives already in the next layer's fp8 scale
    (S0*SW0 = S1*SW1 = S2): the epilogue is then scale-free --
    x~ = max(psum + S*b, 0) -- one 2-op tensor_scalar on DVE (x1) or
    one no-scale Relu activation on ACT (x2). fp8 only has relative
    precision, so per-tensor power-of-2 scales are free; conv1/conv2
    weights ride at scale 1 (some denormals, verified harmless).
  - Maxpool runs on raw PSUM before bias+relu (monotone), on DVE; the
    per-sample partial maxes land in one [*, 2M] tile reduced by a
    single 3D-AP reduce at the end.
  - MPNN in bf16 (total error ~5e-3): aggregation emitted with msg as
    the stationary operand so nei lands directly transposed (no PE
    transposes); molecules processed in PAIRS sharing PSUM banks and
    epilogue instructions (halves the ~300-400ns/instr DVE/ACT
    overhead); W_i/W_o contractions zero-padded to 128 partitions to
    trigger the 4x fast-weight-load path.
  - ~56 tiny warmup matmuls at t=0 ride out the initial DMA wait and
    lift the PE HAM clock-gate to 2.4GHz before real work arrives.
"""

import numpy as np
from contextlib import ExitStack

import concourse.bass as bass
import concourse.tile as tile
from concourse import bacc, mybir
from concourse.bass_utils import run_bass_kernel_spmd

F32 = mybir.dt.float32
BF16 = mybir.dt.bfloat16
FP8 = mybir.dt.float8e4
DR = mybir.MatmulPerfMode.DoubleRow
AF = mybir.ActivationFunctionType
ALU = mybir.AluOpType

B, NA, NB = 128, 48, 96
L = 1000

NCORES = 8
M = B // NCORES          # molecules per core (16)
SEG = 1008               # 3 + 1000 + 5 padded segment (16B-aligned plane)
PAD = 3
NCH = 500                # conv free-dim chunk (2 per sample)

# fp8 power-of-2 scales; S0*SW0 == S1*SW1 == S2 so every conv psum is
# already in its consumer's scale and epilogues need no multiply.
S0 = 32.0                # protein embedding activations
SW0 = 8.0                # conv0 weights
S1 = 256.0               # x1 activations ( = S0*SW0 )
SW1 = 1.0                # conv1 weights
S2 = 256.0               # x2 activations ( = S1*SW1 )
SW2 = 1.0                # conv2 weights
FIN_SCALE = 1.0 / (S2 * SW2)   # 2^-8

_CACHE = {}


def _build_nc():
    nc = bacc.Bacc("TRN2", target_bir_lowering=False, debug=False)

    # ---- DRAM inputs (per core) ----
    d_pvt = [nc.dram_tensor(f"pvt{g}", [50, 2, SEG], FP8, kind="ExternalInput")
             for g in range(M)]
    d_fbt = nc.dram_tensor("fbt", [50, M, 96], BF16, kind="ExternalInput")
    d_cat1 = nc.dram_tensor("cat1", [40, M, 48], BF16, kind="ExternalInput")
    d_abt = nc.dram_tensor("abt", [96, M, 96], BF16, kind="ExternalInput")
    d_aat = nc.dram_tensor("aat", [96, M, 48], BF16, kind="ExternalInput")

    d_wi = nc.dram_tensor("wi", [50, 200], BF16, kind="ExternalInput")
    d_wh = nc.dram_tensor("wh", [100, 2, 200], BF16, kind="ExternalInput")
    d_wo1 = nc.dram_tensor("wo1", [40, 200], BF16, kind="ExternalInput")
    d_wo2 = nc.dram_tensor("wo2", [128, 200], BF16, kind="ExternalInput")
    d_wo3 = nc.dram_tensor("wo3", [72, 200], BF16, kind="ExternalInput")
    d_w0dr = nc.dram_tensor("w0dr", [50, 2, 96], FP8, kind="ExternalInput")
    d_w0s = nc.dram_tensor("w0s", [50, 96], FP8, kind="ExternalInput")
    d_w1dr = nc.dram_tensor("w1dr", [96, 4, 128], FP8, kind="ExternalInput")
    d_w1s = nc.dram_tensor("w1s", [96, 128], FP8, kind="ExternalInput")
    d_w2dra = nc.dram_tensor("w2dra", [128, 6, 128], FP8, kind="ExternalInput")
    d_w2sa = nc.dram_tensor("w2sa", [128, 128], FP8, kind="ExternalInput")
    d_w2drb = nc.dram_tensor("w2drb", [128, 6, 80], FP8, kind="ExternalInput")
    d_w2sb = nc.dram_tensor("w2sb", [128, 72], FP8, kind="ExternalInput")
    d_b0s = nc.dram_tensor("b0s", [96, 1], F32, kind="ExternalInput")
    d_b1s = nc.dram_tensor("b1s", [128, 1], F32, kind="ExternalInput")
    d_b2a = nc.dram_tensor("b2a", [128, 1], F32, kind="ExternalInput")
    d_b2b = nc.dram_tensor("b2b", [72, 1], F32, kind="ExternalInput")
    d_fc0 = [nc.dram_tensor(f"fc0{k}", [dim, 200], BF16, kind="ExternalInput")
             for k, dim in (("a", 128), ("b", 72), ("c", 128), ("d", 72))]
    d_fc0ba = nc.dram_tensor("fc0ba", [128, 1], F32, kind="ExternalInput")
    d_fc0bb = nc.dram_tensor("fc0bb", [72, 1], F32, kind="ExternalInput")
    d_fc1a = nc.dram_tensor("fc1a", [128, 100], BF16, kind="ExternalInput")
    d_fc1b = nc.dram_tensor("fc1b", [72, 100], BF16, kind="ExternalInput")
    d_fc1bias = nc.dram_tensor("fc1bias", [100, 1], F32, kind="ExternalInput")
    d_fc2w = nc.dram_tensor("fc2w", [100, 1], BF16, kind="ExternalInput")
    d_fc2b = nc.dram_tensor("fc2b", [1, 1], F32, kind="ExternalInput")
    d_ones = nc.dram_tensor("ones48", [48, 1], BF16, kind="ExternalInput")

    d_out = nc.dram_tensor("out", [1, M], F32, kind="ExternalOutput")

    with tile.TileContext(nc) as tc, ExitStack() as ctx:
        cst = ctx.enter_context(tc.tile_pool(name="cst", bufs=1))
        sbs = ctx.enter_context(tc.tile_pool(name="sbs", bufs=1))
        tmp = ctx.enter_context(tc.tile_pool(name="tmp", bufs=1))
        xp = ctx.enter_context(tc.tile_pool(name="xp", bufs=1))
        pp = ctx.enter_context(tc.tile_pool(name="pp", bufs=1, space="PSUM"))

        # ---- PE warmup: ride out the initial DMA wait at 1.2GHz so the
        # HAM clock-gate is at 2.4GHz when real matmuls arrive.
        wz = cst.tile([128, 64], BF16, tag="wz")
        nc.gpsimd.memset(wz[:], 0.0)
        for _ in range(56):
            psW = pp.tile([64, 64], F32, tag="tp", bufs=2)
            nc.tensor.matmul(psW[:], wz[:, 0:64], wz[:], start=True, stop=True)

        def const_tile(dram, shape, dtype=BF16, name=None, eng=None):
            t = cst.tile(shape, dtype, tag=name or dram.name)
            (eng or nc.sync).dma_start(t[:], dram.ap())
            return t

        # MPNN inputs arrive in 4 molecule-groups so mol 0 isn't gated on
        # the whole batch. Group g covers mols 4g..4g+3. fbt/cat1 (matmul
        # stationary operands) are zero-padded to 128 partitions -> FWL.
        GM = 4
        fbt_g, abt_g, aat_g, cat1_g = {}, {}, {}, {}

        def fbt_dma(g):
            t = cst.tile([128, GM * 96], BF16, tag=f"fbt{g}")
            nc.vector.memset(t[:], 0.0)
            nc.sync.dma_start(t[0:50, :].rearrange("p (m i) -> p m i", i=96),
                              d_fbt.ap()[:, GM * g:GM * (g + 1), :])
            fbt_g[g] = t

        def abt_dma(g):
            t = cst.tile([96, GM * 96], BF16, tag=f"abt{g}")
            nc.sync.dma_start(t[:].rearrange("p (m i) -> p m i", i=96),
                              d_abt.ap()[:, GM * g:GM * (g + 1), :])
            abt_g[g] = t

        def aat_cat_dma(g, eng):
            t = cst.tile([96, GM * 48], BF16, tag=f"aat{g}")
            eng.dma_start(t[:].rearrange("p (m i) -> p m i", i=48),
                          d_aat.ap()[:, GM * g:GM * (g + 1), :])
            aat_g[g] = t
            t = cst.tile([128, GM * 48], BF16, tag=f"cat1{g}")
            nc.vector.memset(t[:], 0.0)
            eng.dma_start(t[0:40, :].rearrange("p (m i) -> p m i", i=48),
                          d_cat1.ap()[:, GM * g:GM * (g + 1), :])
            cat1_g[g] = t

        # SP queue: mol-group-0 first, then wh / conv2 weights.
        wi_t = cst.tile([128, 200], BF16, tag="wi")
        nc.vector.memset(wi_t[:], 0.0)
        nc.sync.dma_start(wi_t[0:50, :], d_wi.ap())
        fbt_dma(0)
        abt_dma(0)
        wh_t = cst.tile([100, 2 * 200], BF16, tag="wh")
        nc.sync.dma_start(wh_t[:].rearrange("p (c n) -> p c n", n=200),
                          d_wh.ap())
        w2dra_t = cst.tile([128, 6 * 128], FP8, tag="w2dra")
        nc.sync.dma_start(w2dra_t[:].rearrange("p (t o) -> p t o", o=128),
                          d_w2dra.ap())
        w2sa_t = const_tile(d_w2sa, [128, 128], FP8)
        w2drb_t = cst.tile([128, 6 * 80], FP8, tag="w2drb")
        nc.sync.dma_start(w2drb_t[:].rearrange("p (t o) -> p t o", o=80),
                          d_w2drb.ap())
        w2sb_t = const_tile(d_w2sb, [128, 72], FP8)
        aat_cat_dma(0, nc.sync)
        wo1_t = cst.tile([128, 200], BF16, tag="wo1")
        nc.vector.memset(wo1_t[:], 0.0)
        nc.sync.dma_start(wo1_t[0:40, :], d_wo1.ap())
        wo2_t = const_tile(d_wo2, [128, 200], eng=nc.sync)
        wo3_t = const_tile(d_wo3, [72, 200], eng=nc.sync)
        ones_t = const_tile(d_ones, [48, 1], eng=nc.sync)
        b2a_t = const_tile(d_b2a, [128, 1], F32, eng=nc.sync)
        b2b_t = const_tile(d_b2b, [72, 1], F32, eng=nc.sync)
        fbt_dma(1)
        abt_dma(1)
        aat_cat_dma(1, nc.sync)
        fbt_dma(2)
        abt_dma(2)
        aat_cat_dma(2, nc.sync)
        fbt_dma(3)
        abt_dma(3)
        aat_cat_dma(3, nc.sync)
        fc0_t = [const_tile(d, [dim, 200], eng=nc.sync) for d, dim in
                 zip(d_fc0, (128, 72, 128, 72))]
        fc0ba_t = const_tile(d_fc0ba, [128, 1], F32, eng=nc.sync)
        fc0bb_t = const_tile(d_fc0bb, [72, 1], F32, eng=nc.sync)
        fc1a_t = const_tile(d_fc1a, [128, 100], eng=nc.sync)
        fc1b_t = const_tile(d_fc1b, [72, 100], eng=nc.sync)
        fc1bias_t = const_tile(d_fc1bias, [100, 1], F32, eng=nc.sync)
        fc2w_t = const_tile(d_fc2w, [100, 1], eng=nc.sync)
        fc2b_t = const_tile(d_fc2b, [1, 1], F32, eng=nc.sync)

        # ACT queue: conv0 weights + per-sample x0 buffers
        w0dr_t = cst.tile([50, 2 * 96], FP8, tag="w0dr")
        nc.scalar.dma_start(w0dr_t[:].rearrange("p (t o) -> p t o", o=96),
                            d_w0dr.ap())
        w0s_t = const_tile(d_w0s, [50, 96], FP8, eng=nc.scalar)
        b0s_t = const_tile(d_b0s, [96, 1], F32, eng=nc.scalar)
        b1s_t = const_tile(d_b1s, [128, 1], F32, eng=nc.scalar)
        x0_bufs = []

        def pvt_dma(s):
            t = xp.tile([50, 2 * SEG], FP8, tag=f"x0s{s}")
            nc.scalar.dma_start(t[:].rearrange("p (k c) -> p k c", k=2),
                                d_pvt[s].ap())
            x0_bufs.append(t)

        # Pool-engine SWDGE queue: conv1 weights
        w1dr_t = cst.tile([96, 4 * 128], FP8, tag="w1dr")
        nc.gpsimd.dma_start(w1dr_t[:].rearrange("p (t o) -> p t o", o=128),
                            d_w1dr.ap())
        w1s_t = const_tile(d_w1s, [96, 128], FP8, eng=nc.gpsimd)
        pvt_dma(0)
        pvt_dma(1)

        # static outputs of the two towers, feature-major [feat, M]
        embT1 = sbs.tile([128, M], BF16, tag="embT1")
        embT2 = sbs.tile([72, M], BF16, tag="embT2")
        mxAall = sbs.tile([128, 2 * M], F32, tag="mxAall")
        mxBall = sbs.tile([72, 2 * M], F32, tag="mxBall")

        # ============ per-molecule-PAIR MPNN (staged, bf16) ============
        # pair p covers mols (2p, 2p+1); col block j*200 holds mol j's h.
        pair_state = {}

        def emit_binput_pair(p):
            g, r0 = p // 2, 2 * (p % 2)
            psA = pp.tile([96, 400], F32, tag="mp", bufs=3)
            for j in (0, 1):
                fb_m = fbt_g[g][:, (r0 + j) * 96:(r0 + j + 1) * 96]
                nc.tensor.matmul(psA[:, 200 * j:200 * j + 200], fb_m, wi_t[:],
                                 start=True, stop=True)
            binp = sbs.tile([96, 400], F32, tag=f"binp{p}")
            nc.scalar.copy(binp[:], psA[:])
            msg = sbs.tile([96, 400], BF16, tag=f"msg{p}")
            nc.scalar.activation(msg[:], psA[:], AF.Relu)
            pair_state[p] = (binp, msg)

        def emit_iter_pre_pair(p):
            # nei, directly transposed: psT block = msg^T-slice @ A
            g, r0 = p // 2, 2 * (p % 2)
            binp, msg = pair_state[p]
            psT = pp.tile([100, 384], F32, tag="tp", bufs=2)
            for j in (0, 1):
                ab_m = abt_g[g][:, (r0 + j) * 96:(r0 + j + 1) * 96]
                nc.tensor.matmul(psT[:, 192 * j:192 * j + 96],
                                 msg[:, 200 * j:200 * j + 100], ab_m,
                                 start=True, stop=True)
                nc.tensor.matmul(psT[:, 192 * j + 96:192 * j + 192],
                                 msg[:, 200 * j + 100:200 * j + 200], ab_m,
                                 start=True, stop=True)
            nT = tmp.tile([100, 384], BF16, tag="nT", bufs=4)
            nc.scalar.copy(nT[:], psT[:])
            pair_state[p] = (binp, msg, nT)

        def emit_iter_post_pair(p):
            binp, msg, nT = pair_state[p]
            psH = pp.tile([96, 400], F32, tag="mp", bufs=3)
            for j in (0, 1):
                nc.tensor.matmul(psH[:, 200 * j:200 * j + 200],
                                 nT[:, 192 * j:192 * j + 96], wh_t[:, 0:200],
                                 start=True, stop=False)
                nc.tensor.matmul(psH[:, 200 * j:200 * j + 200],
                                 nT[:, 192 * j + 96:192 * j + 192],
                                 wh_t[:, 200:400], start=False, stop=True)
            tm = tmp.tile([96, 400], F32, tag="mtmp", bufs=3)
            nc.vector.tensor_add(tm[:], psH[:], binp[:])
            nc.scalar.activation(msg[:], tm[:], AF.Relu)
            pair_state[p] = (binp, msg)

        def emit_atom_pair(p):
            g, r0 = p // 2, 2 * (p % 2)
            binp, msg = pair_state[p]
            psT1 = pp.tile([128, 96], F32, tag="tp", bufs=2)
            psT2 = pp.tile([72, 96], F32, tag="tp", bufs=2)
            for j in (0, 1):
                aa_m = aat_g[g][:, (r0 + j) * 48:(r0 + j + 1) * 48]
                nc.tensor.matmul(psT1[:, 48 * j:48 * j + 48],
                                 msg[:, 200 * j:200 * j + 128], aa_m,
                                 start=True, stop=True)
                nc.tensor.matmul(psT2[:, 48 * j:48 * j + 48],
                                 msg[:, 200 * j + 128:200 * j + 200], aa_m,
                                 start=True, stop=True)
            nat1 = tmp.tile([128, 96], BF16, tag="nat1", bufs=3)
            nc.scalar.copy(nat1[:], psT1[:])
            nat2 = tmp.tile([72, 96], BF16, tag="nat2", bufs=3)
            nc.scalar.copy(nat2[:], psT2[:])

            psAH = pp.tile([48, 400], F32, tag="mp", bufs=3)
            for j in (0, 1):
                c1_m = cat1_g[g][:, (r0 + j) * 48:(r0 + j + 1) * 48]
                sl = psAH[:, 200 * j:200 * j + 200]
                nc.tensor.matmul(sl, c1_m, wo1_t[:], start=True, stop=False)
                nc.tensor.matmul(sl, nat1[:, 48 * j:48 * j + 48], wo2_t[:],
                                 start=False, stop=False)
                nc.tensor.matmul(sl, nat2[:, 48 * j:48 * j + 48], wo3_t[:],
                                 start=False, stop=True)
            reluh = tmp.tile([48, 400], BF16, tag="reluh", bufs=3)
            nc.scalar.activation(reluh[:], psAH[:], AF.Relu)

            psE1 = pp.tile([128, 2], F32, tag="tp", bufs=2)
            psE2 = pp.tile([72, 2], F32, tag="tp", bufs=2)
            for j in (0, 1):
                nc.tensor.matmul(psE1[:, j:j + 1],
                                 reluh[:, 200 * j:200 * j + 128], ones_t[:],
                                 start=True, stop=True)
                nc.tensor.matmul(psE2[:, j:j + 1],
                                 reluh[:, 200 * j + 128:200 * j + 200],
                                 ones_t[:], start=True, stop=True)
            nc.scalar.mul(embT1[:, 2 * p:2 * p + 2], psE1[:], 1.0 / 48)
            nc.scalar.mul(embT2[:, 2 * p:2 * p + 2], psE2[:], 1.0 / 48)

        # ================= per-sample protein conv tower =================
        # x buffers hold two fp8 planes; plane1 = plane0 shifted one
        # position left, so a DoubleRow AP with base offset o covers the
        # tap pair (x[p+o-PAD], x[p+o-PAD+1]). Data lives at plane0 cols
        # [PAD, PAD+1000); plane1 cols [PAD-1, PAD+999).
        sample_state = {}

        def pads(xflat):
            # flat strips: plane0 head, plane0 tail + plane1 head, plane1 tail
            for a, b in ((0, PAD), (SEG - 5, SEG + PAD - 1),
                         (2 * SEG - 6, 2 * SEG)):
                nc.gpsimd.memset(xflat[:, a:b], 0.0)

        def emit_sample_front(s):
            if s + 2 < M:
                pvt_dma(s + 2)
            x03 = x0_bufs[s][:].rearrange("p (k c) -> p k c", k=2)
            w0dr3 = w0dr_t[:].rearrange("p (k o) -> p k o", o=96)
            x1 = xp.tile([96, 2 * SEG], FP8, tag="x1", bufs=3)
            x13 = x1[:].rearrange("p (k c) -> p k c", k=2)
            pads(x1[:])
            for c in range(2):
                o = c * NCH
                ps = pp.tile([96, NCH], F32, tag="cv", bufs=3)
                nc.tensor.matmul(ps[:], w0dr3, x03[:, :, o + 2:o + 2 + NCH],
                                 start=True, stop=False, perf_mode=DR)
                nc.tensor.matmul(ps[:], w0s_t[:], x03[:, 0, o + 4:o + 4 + NCH],
                                 start=False, stop=True)
                # x~1 = max(psum + S1*b0, 0)  (psum already at scale S1)
                nc.vector.tensor_scalar(x13[:, 0, PAD + o:PAD + o + NCH],
                                        ps[:], b0s_t[:], 0.0,
                                        op0=ALU.add, op1=ALU.max)
                nc.vector.tensor_scalar(x13[:, 1, PAD - 1 + o:PAD - 1 + o + NCH],
                                        ps[:], b0s_t[:], 0.0,
                                        op0=ALU.add, op1=ALU.max)

            w1dr3 = w1dr_t[:].rearrange("p (t o) -> p t o", o=128)
            x2 = xp.tile([128, 2 * SEG], FP8, tag="x2", bufs=4)
            x23 = x2[:].rearrange("p (k c) -> p k c", k=2)
            pads(x2[:])
            for c in range(2):
                o = c * NCH
                ps = pp.tile([128, NCH], F32, tag="cv", bufs=3)
                nc.tensor.matmul(ps[:], w1dr3[:, 0:2, :],
                                 x13[:, :, o + 1:o + 1 + NCH],
                                 start=True, stop=False, perf_mode=DR)
                nc.tensor.matmul(ps[:], w1dr3[:, 2:4, :],
                                 x13[:, :, o + 3:o + 3 + NCH],
                                 start=False, stop=False, perf_mode=DR)
                nc.tensor.matmul(ps[:], w1s_t[:], x13[:, 0, o + 5:o + 5 + NCH],
                                 start=False, stop=True)
                # x~2 = relu(psum + S2*b1), no scale (psum already at S2)
                nc.scalar.activation(x23[:, 0, PAD + o:PAD + o + NCH],
                                     ps[:], AF.Relu, bias=b1s_t[:])
                nc.scalar.activation(x23[:, 1, PAD - 1 + o:PAD - 1 + o + NCH],
                                     ps[:], AF.Relu, bias=b1s_t[:])
            sample_state[s] = x2

        def emit_sample_back(s):
            x2 = sample_state.pop(s)
            x23 = x2[:].rearrange("p (k c) -> p k c", k=2)
            w2dra3 = w2dra_t[:].rearrange("p (t o) -> p t o", o=128)
            w2drb3 = w2drb_t[:].rearrange("p (t o) -> p t o", o=80)
            for c in range(2):
                o = c * NCH
                psA = pp.tile([128, NCH], F32, tag="cv", bufs=3)
                for p in range(3):
                    nc.tensor.matmul(psA[:], w2dra3[:, 2 * p:2 * p + 2, :],
                                     x23[:, :, o + 2 * p:o + 2 * p + NCH],
                                     start=(p == 0), stop=False, perf_mode=DR)
                nc.tensor.matmul(psA[:], w2sa_t[:],
                                 x23[:, 0, o + 6:o + 6 + NCH],
                                 start=False, stop=True)
                nc.vector.reduce_max(mxAall[:, 2 * s + c:2 * s + c + 1], psA[:],
                                     axis=mybir.AxisListType.X)
                psB = pp.tile([72, NCH], F32, tag="cv", bufs=3)
                for p in range(3):
                    nc.tensor.matmul(psB[:], w2drb3[:, 2 * p:2 * p + 2, 0:72],
                                     x23[:, :, o + 2 * p:o + 2 * p + NCH],
                                     start=(p == 0), stop=False, perf_mode=DR)
                nc.tensor.matmul(psB[:], w2sb_t[:],
                                 x23[:, 0, o + 6:o + 6 + NCH],
                                 start=False, stop=True)
                nc.vector.reduce_max(mxBall[:, 2 * s + c:2 * s + c + 1], psB[:],
                                     axis=mybir.AxisListType.X)

        # Stage-interleaved emission: conv stages slot between MPNN
        # stages so the in-order PE stream always has DMA-ready filler.
        for g in range(4):
            prs = [2 * g, 2 * g + 1]
            for p in prs:
                emit_binput_pair(p)
            for p in prs:
                emit_iter_pre_pair(p)
            emit_sample_front(4 * g + 0)
            emit_sample_front(4 * g + 1)
            for p in prs:
                emit_iter_post_pair(p)
            for p in prs:
                emit_iter_pre_pair(p)
            emit_sample_back(4 * g + 0)
            for p in prs:
                emit_iter_post_pair(p)
            emit_sample_front(4 * g + 2)
            emit_sample_back(4 * g + 1)
            for p in prs:
                emit_atom_pair(p)
            emit_sample_front(4 * g + 3)
            emit_sample_back(4 * g + 2)
            emit_sample_back(4 * g + 3)

        # one 3D-AP reduce over all samples' chunk maxes, then the
        # descale+bias+relu epilogue (maxpool-first is exact: monotone)
        prT1p = sbs.tile([128, M], F32, tag="prT1p")
        nc.vector.reduce_max(prT1p[:],
                             mxAall[:].rearrange("p (s c) -> p s c", c=2),
                             axis=mybir.AxisListType.X)
        prT2p = sbs.tile([72, M], F32, tag="prT2p")
        nc.vector.reduce_max(prT2p[:],
                             mxBall[:].rearrange("p (s c) -> p s c", c=2),
                             axis=mybir.AxisListType.X)
        prT1 = sbs.tile([128, M], BF16, tag="prT1")
        nc.scalar.activation(prT1[:], prT1p[:], AF.Relu, bias=b2a_t[:],
                             scale=FIN_SCALE)
        prT2 = sbs.tile([72, M], BF16, tag="prT2")
        nc.scalar.activation(prT2[:], prT2p[:], AF.Relu, bias=b2b_t[:],
                             scale=FIN_SCALE)

        # ================= FC head (bf16) =================
        rhs4 = (embT1, embT2, prT1, prT2)
        ps0a = pp.tile([128, M], F32, tag="tp", bufs=2)
        for k in range(4):
            nc.tensor.matmul(ps0a[:], fc0_t[k][:, 0:128], rhs4[k][:],
                             start=(k == 0), stop=(k == 3))
        h0a = tmp.tile([128, M], BF16, tag="h0a")
        nc.scalar.activation(h0a[:], ps0a[:], AF.Relu, bias=fc0ba_t[:])
        ps0b = pp.tile([72, M], F32, tag="tp", bufs=2)
        for k in range(4):
            nc.tensor.matmul(ps0b[:], fc0_t[k][:, 128:200], rhs4[k][:],
                             start=(k == 0), stop=(k == 3))
        h0b = tmp.tile([72, M], BF16, tag="h0b")
        nc.scalar.activation(h0b[:], ps0b[:], AF.Relu, bias=fc0bb_t[:])

        ps1 = pp.tile([100, M], F32, tag="tp", bufs=2)
        nc.tensor.matmul(ps1[:], fc1a_t[:], h0a[:], start=True, stop=False)
        nc.tensor.matmul(ps1[:], fc1b_t[:], h0b[:], start=False, stop=True)
        h1 = tmp.tile([100, M], BF16, tag="h1")
        nc.scalar.activation(h1[:], ps1[:], AF.Relu, bias=fc1bias_t[:])

        ps2 = pp.tile([1, M], F32, tag="tp", bufs=2)
        nc.tensor.matmul(ps2[:], fc2w_t[:], h1[:], start=True, stop=True)
        outsb = tmp.tile([1, M], F32, tag="outsb")
        nc.scalar.add(outsb[:], ps2[:], fc2b_t[:, 0:1])
        nc.sync.dma_start(d_out.ap(), outsb[:])

    nc.compile()
    return nc


def _prep(inputs):
    """Host preprocessing: returns the 8 per-core in_maps."""
    f32 = np.float32
    fatoms = np.asarray(inputs["fatoms"], f32)
    fbonds = np.asarray(inputs["fbonds"], f32)
    agraph = np.asarray(inputs["agraph"])
    bgraph = np.asarray(inputs["bgraph"])
    pseq = np.asarray(inputs["protein_seq"])
    W_i = np.asarray(inputs["W_i"], f32)
    W_h = np.asarray(inputs["W_h"], f32)
    W_o_w = np.asarray(inputs["W_o_w"], f32)
    W_o_b = np.asarray(inputs["W_o_b"], f32)
    embp = np.asarray(inputs["embed_protein"], f32)

    import ml_dtypes
    bf16 = ml_dtypes.bfloat16
    e4 = ml_dtypes.float8_e4m3fn

    def q8(a, s):
        # TRN fp8_e4m3 matches OCP e4m3fn bit-for-bit up to |x| <= 240
        return np.clip(np.asarray(a, f32) * s, -240, 240).astype(e4)

    # protein embeddings, channel-major
    pvT = np.ascontiguousarray(embp[pseq].transpose(0, 2, 1))  # (B, 50, L)

    # adjacency one-hots (counts; contraction-dim-major for lhsT/rhs use)
    ar = np.arange(B)[:, None, None]
    cntB = np.zeros((B, NB, NB), f32)
    np.add.at(cntB, (ar, np.arange(NB)[None, :, None], bgraph), 1.0)
    abt = np.ascontiguousarray(cntB.transpose(0, 2, 1))        # (B, j, i)
    cntA = np.zeros((B, NA, NB), f32)
    np.add.at(cntA, (ar, np.arange(NA)[None, :, None], agraph), 1.0)
    aat = np.ascontiguousarray(cntA.transpose(0, 2, 1))        # (B, j, a)

    fbT = fbonds.transpose(0, 2, 1)                            # (B, 50, 96)
    faT = fatoms.transpose(0, 2, 1)                            # (B, 39, 48)
    cat1 = np.concatenate([faT, np.ones((B, 1, NA), f32)], axis=1)  # (B,40,48)

    wh = np.zeros((100, 2, 200), f32)
    wh[:, 0, :] = W_h[0:100]
    wh[:, 1, :] = W_h[100:200]
    wo1 = np.zeros((40, 200), f32)
    wo1[:39, :] = W_o_w[0:39]
    wo1[39, :] = W_o_b

    conv_w = [np.asarray(inputs[f"conv{i}_w"], f32) for i in range(3)]
    conv_b = [np.asarray(inputs[f"conv{i}_b"], f32) for i in range(3)]

    # conv weights (out, in, k) -> fp8 tap-pair packs [in, taps, out]
    cw = [q8(conv_w[0], SW0), q8(conv_w[1], SW1), q8(conv_w[2], SW2)]
    w0dr = np.ascontiguousarray(cw[0][:, :, 0:2].transpose(1, 2, 0))
    w0s = np.ascontiguousarray(cw[0][:, :, 2].transpose(1, 0))
    w1dr = np.ascontiguousarray(cw[1][:, :, 0:4].transpose(1, 2, 0))
    w1s = np.ascontiguousarray(cw[1][:, :, 4].transpose(1, 0))
    w2dra = np.ascontiguousarray(cw[2][0:128, :, 0:6].transpose(1, 2, 0))
    w2sa = np.ascontiguousarray(cw[2][0:128, :, 6].transpose(1, 0))
    w2drb = np.zeros((128, 6, 80), e4)
    w2drb[:, :, 0:72] = cw[2][128:200, :, 0:6].transpose(1, 2, 0)
    w2sb = np.ascontiguousarray(cw[2][128:200, :, 6].transpose(1, 0))

    fcw = [np.asarray(inputs[f"fc{i}_w"], f32) for i in range(3)]
    fcb = [np.asarray(inputs[f"fc{i}_b"], f32) for i in range(3)]

    shared_f32 = {
        "b0s": (S1 * conv_b[0]).reshape(96, 1),
        "b1s": (S2 * conv_b[1]).reshape(128, 1),
        "b2a": conv_b[2][0:128].reshape(128, 1),
        "b2b": conv_b[2][128:200].reshape(72, 1),
        "fc0ba": fcb[0][0:128].reshape(128, 1),
        "fc0bb": fcb[0][128:200].reshape(72, 1),
        "fc1bias": fcb[1].reshape(100, 1),
        "fc2b": fcb[2].reshape(1, 1),
    }
    shared_bf16 = {
        "wi": W_i, "wh": wh,
        "wo1": wo1, "wo2": W_o_w[39:167], "wo3": W_o_w[167:239],
        "fc0a": fcw[0][0:128], "fc0b": fcw[0][128:200],
        "fc0c": fcw[0][200:328], "fc0d": fcw[0][328:400],
        "fc1a": fcw[1][0:128], "fc1b": fcw[1][128:200],
        "fc2w": fcw[2], "ones48": np.ones((48, 1), f32),
    }
    shared = {k: np.ascontiguousarray(v, f32) for k, v in shared_f32.items()}
    shared.update({k: np.ascontiguousarray(v, f32).astype(bf16)
                   for k, v in shared_bf16.items()})
    shared.update({"w0dr": w0dr, "w0s": w0s, "w1dr": w1dr, "w1s": w1s,
                   "w2dra": w2dra, "w2sa": w2sa, "w2drb": w2drb,
                   "w2sb": w2sb})

    # protein activations: fp8 two-plane layout with conv pads baked in
    # plane0[:, PAD+q] = x[q]; plane1[:, PAD-1+q] = x[q]
    pvq = q8(pvT, S0)                                          # (B, 50, L)
    pvt_pad = np.zeros((B, 50, 2, SEG), e4)
    pvt_pad[:, :, 0, PAD:PAD + L] = pvq
    pvt_pad[:, :, 1, PAD - 1:PAD - 1 + L] = pvq

    in_maps = []
    for c in range(NCORES):
        lo = c * M
        im = dict(shared)
        for g in range(M):
            im[f"pvt{g}"] = np.ascontiguousarray(pvt_pad[lo + g])
        im["fbt"] = np.ascontiguousarray(
            fbT[lo:lo + M].transpose(1, 0, 2)).astype(bf16)
        im["cat1"] = np.ascontiguousarray(
            cat1[lo:lo + M].transpose(1, 0, 2)).astype(bf16)
        im["abt"] = np.ascontiguousarray(
            abt[lo:lo + M].transpose(1, 0, 2)).astype(bf16)
        im["aat"] = np.ascontiguousarray(
            aat[lo:lo + M].transpose(1, 0, 2)).astype(bf16)
        in_maps.append(im)
    return in_maps


def get_nc():
    if "nc" not in _CACHE:
        _CACHE["nc"] = _build_nc()
    return _CACHE["nc"]


def kernel(**inputs) -> np.ndarray:
    nc = get_nc()
    in_maps = _prep(inputs)
    res = run_bass_kernel_spmd(nc, in_maps, core_ids=list(range(NCORES)))
    outs = [res.results[c]["out"].reshape(M, 1) for c in range(NCORES)]
    return np.concatenate(outs, axis=0).astype(np.float32)
